# revision 18
# baseline (speedup 1.0000x reference)
"""BiWKV6 encoder kernel for 8 Trainium2 NeuronCores.

Sharding: (batch, direction) -> 8 units, one per core; core c handles
batch c % 4, direction c // 4. Backward cores run the identical SPMD
program on time-reversed inputs; the only cross-core communication is a
pairwise AllGather of each layer's block output, written time-reversed
into the partner's domain. Within a core activations are channel-major
[C, T]; the WKV scan uses the chunked linear-attention formulation
(chunk 128) with log-space cumulative decay from the DVE prefix scan.

v4: software-pipelined across the two 512-token frames — the serial
WKV recurrence of frame 0 is chunk-interleaved with frame 1's
precompute, and frame 1's recurrence with frame 0's GroupNorm stats, so
the latency-bound S chain always has independent work beside it in the
in-order engine queues.  maa folded into the shift matmuls (ones-row),
bf16 token-shift pipeline (DVE 2x/4x fast modes), relu^2 on the ACT
engine, tanh-form gates (tanh shares the exp activation table, so the
only ACT table reloads are at the Ln clusters, which are paired across
frames), GroupNorm stats via segmented tensor_reduce with stride-0
broadcast normalization, premasked intra-chunk attention matrices.
"""
import numpy as np

import concourse.bass as bass
import concourse.tile as tile
from concourse import bacc, mybir
from concourse.bass_utils import run_bass_kernel_spmd

B, T, C = 4, 1024, 512
H, HN = 8, 64
L = 128
TTW = 512
NTT = T // TTW
NCPT = TTW // L
CT = C // 128
TM, TD, FFN, NL = 32, 64, 1792, 2
NFF = FFN // 128
EPS_LN, EPS_GN = 1e-5, 64e-5
NV = 19
TME = TM + 1  # tm_w2 rows + folded maa row

F32 = mybir.dt.float32
F32R = mybir.dt.float32r
BF16 = mybir.dt.bfloat16
AF = mybir.ActivationFunctionType
OP = mybir.AluOpType
AX = mybir.AxisListType

WB_R, WB_K, WB_V, WB_G, WB_TM1, WB_TD1, WB_O = 0, 512, 1024, 1536, 2048, 2208, 2272
WB_COLS = 2784
(V_LN1W, V_LN1B, V_LN2W, V_LN2B, V_MAAX, V_MAAW, V_MAAK, V_MAAV, V_MAAR,
 V_MAAG, V_TDCY, V_CMK, V_CMR, V_GBM, V_CW0, V_CW1, V_CW2, V_LN0W,
 V_LN0B) = range(NV)
S_LN0, S_NEGS, S_ALPHA, S_BETA = 0, 1, 2, 3

_CACHE = {}


def _revap(ap):
    n = ap.ap[-1][1]
    return bass.AP(tensor=ap.tensor, offset=ap.offset + (n - 1) * ap.ap[-1][0],
                   ap=[ap.ap[0], [-ap.ap[-1][0], n]])


def _bcast(t, off, n, rep):
    # [128, n] slice starting at free-offset `off`, each column repeated
    # `rep` times via a stride-0 inner AP dim
    return bass.AP(tensor=t.tensor, offset=t.offset + off,
                   ap=[t.ap[0], [1, n], [0, rep]])


def _build(dbg=False, solo=False):
    nc = bacc.Bacc("TRN2", target_bir_lowering=False, debug=False, num_devices=8)

    x0 = nc.declare_dram_parameter("x0", [C, T], F32, isOutput=False)
    x1 = nc.declare_dram_parameter("x1", [C, T], F32, isOutput=False)
    mask05 = nc.declare_dram_parameter("mask05", [128, T], BF16, isOutput=False)
    sel_in = nc.declare_dram_parameter("sel", [128, 8], F32, isOutput=False)
    trib_in = nc.declare_dram_parameter("trib", [128, 128], BF16, isOutput=False)
    wbig, tmw2, tdw2, cmkp, cmvp, cmrg, vecs_in, lnx_in, hmu_in = \
        [], [], [], [], [], [], [], [], []
    identh_in = nc.declare_dram_parameter("identh", [128, 128], BF16, isOutput=False)
    for l in range(NL):
        wbig.append(nc.declare_dram_parameter(f"wbig{l}", [C, WB_COLS], BF16, isOutput=False))
        tmw2.append(nc.declare_dram_parameter(f"tmw2{l}", [5 * TME, C], BF16, isOutput=False))
        tdw2.append(nc.declare_dram_parameter(f"tdw2{l}", [TD, C], BF16, isOutput=False))
        cmkp.append(nc.declare_dram_parameter(f"cmk{l}", [NFF, C, 128], BF16, isOutput=False))
        cmvp.append(nc.declare_dram_parameter(f"cmv{l}", [FFN, C], BF16, isOutput=False))
        cmrg.append(nc.declare_dram_parameter(f"cmrg{l}", [C, 1024], BF16, isOutput=False))
        vecs_in.append(nc.declare_dram_parameter(f"vecs{l}", [C, NV], F32, isOutput=False))
        lnx_in.append(nc.declare_dram_parameter(f"lnx{l}", [128, 1024], BF16, isOutput=False))
        hmu_in.append(nc.declare_dram_parameter(f"hmu{l}", [C, 8], F32, isOutput=False))
    xout = nc.declare_dram_parameter("xout", [C, T], F32, isOutput=True)

    groups = [[0, 4], [1, 5], [2, 6], [3, 7]]

    with tile.TileContext(nc) as tc:
        with (
            tc.tile_pool(name="pp", bufs=1) as pp,
            tc.tile_pool(name="wp", bufs=2) as wp,
            tc.tile_pool(name="kp", bufs=1) as kp,
            tc.tile_pool(name="k2", bufs=2) as k2,
            tc.tile_pool(name="psA", bufs=1, space="PSUM") as psA,
            tc.tile_pool(name="psB", bufs=2, space="PSUM") as psB,
            tc.tile_pool(name="psC", bufs=1, space="PSUM") as psC,
            tc.tile_pool(name="dp", bufs=2, space="DRAM") as dp,
        ):
            # ------------- persistent loads -------------
            xres = [pp.tile([128, T], F32R, tag=f"xres{i}", name=f"xres{i}") for i in range(CT)]
            xb = [pp.tile([128, T], F32R, tag=f"xb{i}", name=f"xb{i}") for i in range(CT)]
            for i in range(CT):
                nc.sync.dma_start(out=xres[i],
                                  in_=x0[i * 128:(i + 1) * 128, :].bitcast(F32R))
                nc.sync.dma_start(out=xb[i],
                                  in_=x1[i * 128:(i + 1) * 128, :].bitcast(F32R))
            maskt = pp.tile([128, T], BF16, tag="mask", name="mask")
            nc.sync.dma_start(out=maskt, in_=mask05[:, :])
            selt = pp.tile([128, 8], F32, tag="sel", name="sel")
            nc.sync.dma_start(out=selt, in_=sel_in[:, :])
            eps_ln_t = pp.tile([128, 1], F32, tag="epsln", name="epsln")
            nc.vector.memset(eps_ln_t, EPS_LN)
            eps_gn_t = pp.tile([128, 1], F32, tag="epsgn", name="epsgn")
            nc.vector.memset(eps_gn_t, EPS_GN)
            triu = pp.tile([128, 128], BF16, tag="triu", name="triu")
            nc.sync.dma_start(out=triu, in_=trib_in[:, :])
            idh = pp.tile([128, 128], BF16, tag="idh", name="idh")
            nc.sync.dma_start(out=idh, in_=identh_in[:, :])
            onesr_t = pp.tile([128, 128], F32R, tag="onesr", name="onesr")
            nc.vector.memset(onesr_t.bitcast(F32), 1.0)
            ones_r = onesr_t

            def vcol(vt, i, j):
                return vt[i][:, j:j + 1]

            def load_w(dram_ap, shape, tag, bufs=2):
                t = wp.tile(shape, BF16, tag=tag, name="wld", bufs=bufs)
                nc.sync.dma_start(out=t, in_=dram_ap)
                return t

            def load_wblk(dram_2d, shape):
                # one DMA for a [C, w] weight block -> SBUF [128, CT, w]
                t = wp.tile(shape, BF16, tag="wblk", name="wblk", bufs=2)
                nc.sync.dma_start(
                    out=t, in_=dram_2d.rearrange("(k p) n -> p k n", p=128))
                return t

            # LN in three phases so the Ln / Exp ops of several frames can be
            # emitted adjacently (each Ln->Exp family switch reloads the ACT
            # function table at ~1.3us)
            def ln_pre(src_sl):
                ssum = psC.tile([1, TTW], F32, tag="stA", name="stA")
                ssq = psC.tile([1, TTW], F32, tag="stB", name="stB")
                for i in range(CT):
                    sq = k2.tile([128, TTW], F32R, tag="lnsq", name="lnsq",
                                 bufs=1)
                    nc.gpsimd.tensor_mul(out=sq,
                                         in0=src_sl[i].bitcast(F32),
                                         in1=src_sl[i].bitcast(F32))
                    nc.tensor.matmul(out=ssum, lhsT=ones_r[:, 0:1], rhs=src_sl[i],
                                     start=(i == 0), stop=(i == CT - 1))
                    nc.tensor.matmul(out=ssq, lhsT=ones_r[:, 0:1], rhs=sq,
                                     start=(i == 0), stop=(i == CT - 1))
                rows = k2.tile([128, TTW], F32, tag="lnrows", name="lnrows",
                               bufs=2)
                srow, s2, varu = (rows[j:j + 1, :] for j in (0, 32, 64))
                nc.scalar.activation(out=srow, in_=ssum, func=AF.Copy)
                nc.vector.tensor_mul(out=s2, in0=srow, in1=srow)
                nc.vector.scalar_tensor_tensor(out=varu, in0=s2, scalar=-1.0 / C,
                                               in1=ssq, op0=OP.mult, op1=OP.add)
                return rows

            def ln_ln(rows):
                nc.scalar.activation(out=rows[96:97, :], in_=rows[64:65, :],
                                     func=AF.Ln, scale=1.0 / C,
                                     bias=eps_ln_t[0:1, :])

            def ln_exp(rows):
                rs = k2.tile([1, TTW], F32R, tag="lnrs", name="lnrs", bufs=2)
                nc.scalar.activation(out=rs, in_=rows[96:97, :], func=AF.Exp,
                                     scale=-0.5)
                return rs

            def ln_apply(rows, rs, src_sl, vt, wi, bi, out_tiles):
                murs = k2.tile([1, TTW], F32R, tag="lnmu", name="lnmu", bufs=1)
                nc.vector.scalar_tensor_tensor(out=murs, in0=rows[0:1, :],
                                               scalar=1.0 / C, in1=rs,
                                               op0=OP.mult, op1=OP.mult)
                bc0 = psB.tile([128, TTW], F32, tag="pw", name="pw")
                nc.tensor.matmul(out=bc0, lhsT=ones_r[0:1, 0:128], rhs=rs,
                                 start=True, stop=True)
                bc1 = psB.tile([128, TTW], F32, tag="pw", name="pw")
                nc.tensor.matmul(out=bc1, lhsT=ones_r[0:1, 0:128], rhs=murs,
                                 start=True, stop=True)
                for i in range(CT):
                    t1 = k2.tile([128, TTW], F32, tag="lnt1", name="lnt1")
                    nc.vector.tensor_mul(out=t1, in0=src_sl[i], in1=bc0)
                    nc.vector.tensor_sub(out=t1, in0=t1, in1=bc1)
                    nc.scalar.activation(out=out_tiles[i], in_=t1, func=AF.Identity,
                                         scale=vcol(vt, i, wi),
                                         bias=vcol(vt, i, bi))

            def ln_multi(srcs):
                rows_l = [ln_pre(s) for s in srcs]
                for r in rows_l:
                    ln_ln(r)
                rs_l = [ln_exp(r) for r in rows_l]
                return rows_l, rs_l

            def tanh_act(psum_ap, out_tile, scale=1.0):
                nc.scalar.activation(out=out_tile, in_=psum_ap, func=AF.Tanh,
                                     scale=scale)

            # ================= layers =================
            for l in range(NL):
                vecs = []
                for i in range(CT):
                    vt = pp.tile([128, NV], F32, tag=f"vecs{i}", name=f"vecs{i}")
                    nc.sync.dma_start(out=vt, in_=vecs_in[l][i * 128:(i + 1) * 128, :])
                    vecs.append(vt)
                lnxt = pp.tile([128, 1024], BF16, tag="lnx", name="lnx")
                nc.sync.dma_start(out=lnxt, in_=lnx_in[l][:, :])
                hmu = []
                for i in range(CT):
                    ht = pp.tile([128, 8], F32, tag=f"hmu{i}", name=f"hmu{i}")
                    nc.sync.dma_start(out=ht, in_=hmu_in[l][i * 128:(i + 1) * 128, :])
                    hmu.append(ht)

                # xb init for l==0 comes precomputed from the host (x1);
                # for l>0, xb is initialized lazily: time-mix LN reads xres
                # directly and the Wo accumulation writes xb = xres + Wo@y.

                S_box = [pp.tile([128, HN], BF16, tag=f"S{i}", name=f"S{i}") for i in range(CT)]
                for i in range(CT):
                    nc.vector.memset(S_box[i], 0.0)
                S_box = [S_box]  # boxed so stage_c can rebind
                carry = [pp.tile([128, 1], F32, tag=f"ca{i}", name=f"ca{i}") for i in range(CT)]
                carry2 = [pp.tile([128, 1], F32, tag=f"cb{i}", name=f"cb{i}") for i in range(CT)]
                for i in range(CT):
                    nc.gpsimd.memset(carry[i], 0.0)
                    nc.gpsimd.memset(carry2[i], 0.0)

                # ================= time mix =================
                src_res = xb if l == 0 else xres
                tm_srcs = [[src_res[i][:, tt * TTW:(tt + 1) * TTW]
                            for i in range(CT)] for tt in range(NTT)]
                tm_rows, tm_rs = ln_multi(tm_srcs)

                def stage_a(tt):
                    st = {"tt": tt, "sl": slice(tt * TTW, (tt + 1) * TTW)}
                    xt = [kp.tile([128, TTW], BF16, tag=f"xt{i}", name=f"xt{i}") for i in range(CT)]
                    ln_apply(tm_rows[tt], tm_rs[tt], tm_srcs[tt], vecs,
                             V_LN1W, V_LN1B, xt)
                    xx = [kp.tile([128, TTW], BF16, tag=f"xx{i}", name=f"xx{i}") for i in range(CT)]
                    for i in range(CT):
                        nc.vector.tensor_sub(out=xx[i][:, 1:TTW],
                                             in0=xt[i][:, 0:TTW - 1],
                                             in1=xt[i][:, 1:TTW])
                        nc.vector.scalar_tensor_tensor(
                            out=xx[i][:, 0:1], in0=carry[i], scalar=1.0,
                            in1=xt[i][:, 0:1], op0=OP.mult, op1=OP.subtract)
                        nc.gpsimd.tensor_copy(out=carry[i], in_=xt[i][:, TTW - 1:TTW])

                    # ---- t5 ----
                    mx = [k2.tile([128, TTW], BF16, tag=f"xf{i}", name=f"xf{i}") for i in range(CT)]
                    for i in range(CT):
                        xxm = k2.tile([128, TTW], BF16, tag="xxm", name="xxm", bufs=1)
                        nc.vector.tensor_scalar_mul(out=xxm, in0=xx[i],
                                                    scalar1=vcol(vecs, i, V_MAAX))
                        nc.vector.tensor_add(out=mx[i], in0=xxm, in1=xt[i])
                    p160a = psC.tile([128, TTW], F32, tag="stA", name="stA")
                    p160b = psC.tile([32, TTW], F32, tag="stB", name="stB")
                    wtmtd = wp.tile([128, CT, 224], BF16, tag="wtmtd", name="wtmtd",
                                    bufs=1)
                    nc.sync.dma_start(
                        out=wtmtd,
                        in_=wbig[l][:, WB_TM1:WB_TM1 + 224]
                        .rearrange("(k p) n -> p k n", p=128))
                    for i in range(CT):
                        nc.tensor.matmul(out=p160a, lhsT=wtmtd[:, i, 0:128], rhs=mx[i],
                                         start=(i == 0), stop=(i == CT - 1))
                        nc.tensor.matmul(out=p160b, lhsT=wtmtd[:, i, 128:160], rhs=mx[i],
                                         start=(i == 0), stop=(i == CT - 1))
                    # t5 tiles carry an extra all-ones row so the folded maa
                    # row of tmw2 lands as a bias in the shift matmuls
                    t5 = [k2.tile([TME, TTW], BF16, tag=f"t5{f}", name=f"t5{f}", bufs=1) for f in range(5)]
                    for f in range(5):
                        nc.vector.memset(t5[f][TM:TME, :], 1.0)
                    for f in range(4):
                        tanh_act(p160a[f * 32:(f + 1) * 32, :], t5[f][0:TM, :])
                    tanh_act(p160b, t5[4][0:TM, :])

                    def build_xf(fidx):
                        w2 = load_w(tmw2[l][fidx * TME:(fidx + 1) * TME, :],
                                    [TME, C], "wtm2")
                        xft = []
                        for i in range(CT):
                            dlp = psB.tile([128, TTW], F32, tag="pw", name="pw")
                            nc.tensor.matmul(out=dlp,
                                             lhsT=w2[:, i * 128:(i + 1) * 128],
                                             rhs=t5[fidx], start=True, stop=True)
                            a = k2.tile([128, TTW], BF16, tag="xfa", name="xfa")
                            nc.scalar.activation(out=a, in_=dlp, func=AF.Copy)
                            nc.vector.tensor_mul(out=a, in0=a, in1=xx[i])
                            xf = k2.tile([128, TTW], BF16, tag=f"xf{i}", name=f"xf{i}")
                            nc.vector.tensor_add(out=xf, in0=a, in1=xt[i])
                            xft.append(xf)
                        return xft

                    def wmm(col_off, xft):
                        accs = [psA.tile([128, TTW], F32, tag=f"acc{m}",
                                         name=f"acc{m}") for m in range(4)]
                        wt = load_wblk(wbig[l][:, col_off:col_off + 512],
                                       [128, CT, 512])
                        for i in range(CT):
                            for m in range(4):
                                nc.tensor.matmul(out=accs[m],
                                                 lhsT=wt[:, i, m * 128:(m + 1) * 128],
                                                 rhs=xft[i], start=(i == 0),
                                                 stop=(i == CT - 1))
                        return accs

                    def wmm_tm(col_off, xft):
                        accs = [psA.tile([128, TTW], F32, tag=f"acc{m}",
                                         name=f"acc{m}") for m in range(4)]
                        wt = load_wblk(wbig[l][:, col_off:col_off + 512],
                                       [128, CT, 512])
                        for i in range(CT):
                            for ci in range(NCPT):
                                nc.tensor.matmul(out=accs[ci],
                                                 lhsT=xft[i][:, ci * L:(ci + 1) * L],
                                                 rhs=wt[:, i, :], start=(i == 0),
                                                 stop=(i == CT - 1))
                        return accs

                    # k
                    xf = build_xf(1)
                    accs = wmm(WB_K, xf)
                    k_sb = [kp.tile([128, TTW], BF16, tag=f"ksb{i}", name=f"ksb{i}") for i in range(CT)]
                    for m in range(4):
                        nc.scalar.activation(out=k_sb[m], in_=accs[m], func=AF.Copy)
                    # v token-major
                    xf = build_xf(2)
                    accs = wmm_tm(WB_V, xf)
                    v_tm = [kp.tile([128, C], BF16, tag=f"vtm{tt}{ci}", name=f"vtm{tt}{ci}") for ci in range(NCPT)]
                    for ci in range(NCPT):
                        nc.scalar.activation(out=v_tm[ci], in_=accs[ci], func=AF.Copy)
                    # r
                    xf = build_xf(3)
                    accs = wmm(WB_R, xf)
                    r_sb = [kp.tile([128, TTW], BF16, tag=f"rsb{i}", name=f"rsb{i}") for i in range(CT)]
                    for m in range(4):
                        nc.scalar.activation(out=r_sb[m], in_=accs[m], func=AF.Copy)
                    # g token-major: 2*silu(x) = x*(1+tanh(x/2)); the 0.5 is
                    # folded into lnx_w/lnx_b on the host
                    xf = build_xf(4)
                    accs = wmm_tm(WB_G, xf)
                    g_tm = [kp.tile([128, C], BF16, tag=f"gtm{tt}{ci}", name=f"gtm{tt}{ci}") for ci in range(NCPT)]
                    for ci in range(NCPT):
                        e = k2.tile([128, C], BF16, tag="gte", name="gte", bufs=1)
                        tanh_act(accs[ci], e, scale=0.5)
                        nc.vector.tensor_scalar_add(out=e, in0=e, scalar1=1.0)
                        nc.vector.tensor_mul(out=g_tm[ci], in0=e, in1=accs[ci])
                    # w -> wacc -> lai
                    xf = build_xf(0)
                    tdp = psC.tile([TD, TTW], F32, tag="stA", name="stA")
                    for i in range(CT):
                        nc.tensor.matmul(out=tdp, lhsT=wtmtd[:, i, 160:224], rhs=xf[i],
                                         start=(i == 0), stop=(i == CT - 1))
                    tdt = k2.tile([TD, TTW], BF16, tag="tdt", name="tdt", bufs=1)
                    tanh_act(tdp, tdt)
                    w2t = load_w(tdw2[l][:, :], [TD, C], "wtd2", bufs=1)
                    lai = [kp.tile([128, 1 + TTW], F32, tag=f"lai{i}", name=f"lai{i}") for i in range(CT)]
                    for i in range(CT):
                        wwp = psB.tile([128, TTW], F32, tag="pw", name="pw")
                        nc.tensor.matmul(out=wwp, lhsT=w2t[:, i * 128:(i + 1) * 128],
                                         rhs=tdt, start=True, stop=True)
                        wacc = k2.tile([128, TTW], F32, tag="lnt1", name="lnt1")
                        nc.scalar.activation(out=wacc, in_=wwp, func=AF.Exp,
                                             bias=vcol(vecs, i, V_TDCY))
                        nc.gpsimd.memset(lai[i][:, 0:1], 0.0)
                        nc.vector.tensor_tensor_scan(
                            out=lai[i][:, 1:1 + TTW], data0=wacc, data1=wacc,
                            initial=0.0, op0=OP.add, op1=OP.bypass)
                    st.update(k_sb=k_sb, r_sb=r_sb, v_tm=v_tm, g_tm=g_tm, lai=lai,
                              fn_a=[[None] * CT for _ in range(NCPT)],
                              rt_a=[[None] * CT for _ in range(NCPT)],
                              khtm_a=[[None] * CT for _ in range(NCPT)],
                              pts_a=[[[None] * 2 for _ in range(CT)]
                                     for _ in range(NCPT)],
                              dall_a=[None] * NCPT, ysb_a=[None] * NCPT)
                    return st

                # per-chunk precompute: everything that does not depend on the
                # serial S recurrence, incl. the premasked intra-chunk
                # attention matrices
                def stage_b(st, ci):
                    c0 = ci * L
                    lai, k_sb, r_sb = st["lai"], st["k_sb"], st["r_sb"]
                    mt_c = []
                    for i in range(CT):
                        ngc = k2.tile([128, 1], F32, tag="ngc", name="ngc", bufs=4)
                        nc.vector.tensor_scalar_mul(out=ngc,
                                                    in0=lai[i][:, c0:c0 + 1],
                                                    scalar1=-1.0)
                        fp = k2.tile([128, 1 + L], BF16, tag="fp", name="fp",
                                     bufs=2)
                        nc.scalar.activation(out=fp, in_=lai[i][:, c0:c0 + 1 + L],
                                             func=AF.Exp, bias=ngc)
                        fn = k2.tile([128, 1 + L], BF16, tag="fn", name="fn",
                                     bufs=2)
                        nc.scalar.activation(out=fn, in_=lai[i][:, c0:c0 + 1 + L],
                                             func=AF.Exp, scale=-1.0,
                                             bias=lai[i][:, c0:c0 + 1])
                        fnl = k2.tile([128, 1], F32, tag="fnl", name="fnl",
                                      bufs=NCPT * CT)
                        nc.gpsimd.tensor_copy(out=fnl, in_=fn[:, L:L + 1])
                        st["fn_a"][ci][i] = fnl
                        rt = k2.tile([128, L], BF16, tag="rt", name="rt",
                                     bufs=NCPT * CT)
                        nc.gpsimd.tensor_mul(out=rt, in0=r_sb[i][:, c0:c0 + L],
                                             in1=fn[:, 0:L])
                        kt = k2.tile([128, L], BF16, tag="kt", name="kt",
                                     bufs=2)
                        nc.gpsimd.tensor_mul(out=kt, in0=k_sb[i][:, c0:c0 + L],
                                             in1=fp[:, 1:1 + L])
                        kh = k2.tile([128, L], BF16, tag="kh", name="kh", bufs=2)
                        nc.vector.tensor_scalar_mul(out=kh, in0=kt, scalar1=fnl)
                        mt = k2.tile([128, L], F32, tag="mt", name="mt", bufs=4)
                        nc.gpsimd.tensor_mul(out=mt, in0=r_sb[i][:, c0:c0 + L],
                                             in1=k_sb[i][:, c0:c0 + L])
                        mt_c.append(mt)
                        st["rt_a"][ci][i] = rt
                        trp = psB.tile([128, L], BF16, tag="pw", name="pw")
                        nc.tensor.transpose(out=trp, in_=kh, identity=idh)
                        kht = k2.tile([128, L], BF16, tag="khtm", name="khtm",
                                      bufs=NCPT * CT)
                        nc.scalar.activation(out=kht, in_=trp, func=AF.Copy)
                        st["khtm_a"][ci][i] = kht
                        for hh in range(2):
                            hb = hh * HN
                            pt = psB.tile([L, L], F32, tag="pw", name="pw")
                            nc.tensor.matmul(out=pt, lhsT=kt[hb:hb + HN, :],
                                             rhs=rt[hb:hb + HN, :],
                                             start=True, stop=True)
                            pts = k2.tile([L, L], BF16, tag="pts", name="pts",
                                          bufs=2 * NCPT * CT)
                            nc.vector.tensor_mul(out=pts, in0=pt, in1=triu)
                            st["pts_a"][ci][i][hh] = pts
                    dall = psC.tile([128, 8], F32, tag="stB", name="stB")
                    for i in range(CT):
                        nc.tensor.matmul(out=dall, lhsT=mt_c[i], rhs=hmu[i],
                                         start=(i == 0), stop=(i == CT - 1))
                    dsb = k2.tile([128, 8], F32, tag="dsb", name="dsb",
                                  bufs=NCPT)
                    nc.scalar.activation(out=dsb, in_=dall, func=AF.Copy)
                    st["dall_a"][ci] = dsb

                # one chunk of the serial S recurrence (gn deferred)
                def stage_c(st, ci):
                    tt = st["tt"]
                    gc = tt * NCPT + ci
                    rt_t, kh_tm = st["rt_a"][ci], st["khtm_a"][ci]
                    v_tm = st["v_tm"]
                    S_cur = S_box[0]
                    yps = psC.tile([128, C], F32, tag="stA", name="yps")
                    S_new = [k2.tile([128, HN], BF16, tag=f"Sn{i}", name=f"Sn{i}") for i in range(CT)]
                    for i in range(CT):
                        sup = psC.tile([128, HN], F32, tag="stB", name="sup")
                        for hh in range(2):
                            h = 2 * i + hh
                            hb = hh * HN
                            nc.tensor.matmul(
                                out=yps[:, h * HN:(h + 1) * HN],
                                lhsT=st["pts_a"][ci][i][hh],
                                rhs=v_tm[ci][:, h * HN:(h + 1) * HN],
                                start=True, stop=(gc == 0), skip_group_check=True)
                            if gc > 0:
                                nc.tensor.matmul(
                                    out=yps[:, h * HN:(h + 1) * HN],
                                    lhsT=rt_t[i][hb:hb + HN, :],
                                    rhs=S_cur[i][hb:hb + HN, :],
                                    start=False, stop=True, skip_group_check=True)
                            nc.tensor.matmul(
                                out=sup[hb:hb + HN, :],
                                lhsT=kh_tm[i][:, hb:hb + HN],
                                rhs=v_tm[ci][:, h * HN:(h + 1) * HN],
                                start=True, stop=True, skip_group_check=True)
                        nc.vector.scalar_tensor_tensor(
                            out=S_new[i], in0=S_cur[i], scalar=st["fn_a"][ci][i],
                            in1=sup, op0=OP.mult, op1=OP.add)
                    S_box[0] = S_new
                    # u-term: ysb = v * dall_bcast + yps
                    tmpv = k2.tile([128, C], BF16, tag="ytmp", name="ytmp", bufs=1)
                    nc.vector.tensor_mul(out=tmpv, in0=v_tm[ci],
                                         in1=_bcast(st["dall_a"][ci], 0, 8, HN))
                    ysb = k2.tile([128, C], BF16, tag=f"ysb{tt}{ci}",
                                  name=f"ysb{tt}{ci}", bufs=1)
                    nc.vector.tensor_add(out=ysb, in0=tmpv, in1=yps)
                    st["ysb_a"][ci] = ysb

                # groupnorm stats for one chunk (segmented tensor_reduce)
                def stage_dstats(st, ci):
                    tt = st["tt"]
                    if ci == 0:
                        st["mu_all"] = k2.tile([128, 8 * NCPT], F32,
                                               tag=f"gnmu{tt}", name=f"gnmu{tt}")
                        st["var_all"] = k2.tile([128, 8 * NCPT], F32,
                                                tag=f"gnvar{tt}", name=f"gnvar{tt}")
                    ysb = st["ysb_a"][ci]
                    sqt = k2.tile([128, C], BF16, tag="gnsq", name="gnsq", bufs=1)
                    nc.scalar.activation(out=sqt, in_=ysb, func=AF.Square)
                    suv = k2.tile([128, 8], F32, tag="gnsu", name="gnsu")
                    yv = bass.AP(tensor=ysb.tensor, offset=ysb.offset,
                                 ap=[ysb.ap[0], [HN, 8], [1, HN]])
                    nc.vector.tensor_reduce(out=suv, in_=yv, axis=AX.X, op=OP.add)
                    sqv = k2.tile([128, 8], F32, tag="gnsv", name="gnsv")
                    qv = bass.AP(tensor=sqt.tensor, offset=sqt.offset,
                                 ap=[sqt.ap[0], [HN, 8], [1, HN]])
                    nc.vector.tensor_reduce(out=sqv, in_=qv, axis=AX.X, op=OP.add)
                    mu = st["mu_all"][:, 8 * ci:8 * ci + 8]
                    nc.vector.tensor_scalar_mul(out=mu, in0=suv, scalar1=1.0 / HN)
                    msq = k2.tile([128, 8], F32, tag="gnms", name="gnms")
                    nc.vector.tensor_mul(out=msq, in0=mu, in1=mu)
                    nc.vector.scalar_tensor_tensor(
                        out=st["var_all"][:, 8 * ci:8 * ci + 8], in0=sqv,
                        scalar=1.0 / HN, in1=msq, op0=OP.mult, op1=OP.subtract)

                def stage_gn_ln(st):
                    lnv = k2.tile([128, 8 * NCPT], F32, tag="gnln", name="gnln")
                    nc.scalar.activation(out=lnv, in_=st["var_all"][:, :],
                                         func=AF.Ln, bias=eps_gn_t)
                    st["lnv"] = lnv

                def stage_gn_exp(st):
                    rsg = k2.tile([128, 8 * NCPT], BF16, tag="gnrs", name="gnrs")
                    nc.scalar.activation(out=rsg, in_=st["lnv"], func=AF.Exp,
                                         scale=-0.5)
                    st["rsg"] = rsg

                # normalize + affine + *g + transpose into ztc, then Wo
                def stage_dnorm_wo(st):
                    tt = st["tt"]
                    sl = st["sl"]
                    ztc = [kp.tile([128, TTW], BF16, tag=f"ztc{i}", name=f"ztc{i}") for i in range(CT)]
                    for ci in range(NCPT):
                        c0 = ci * L
                        ysb = st["ysb_a"][ci]
                        ysn = k2.tile([128, C], BF16, tag="gnd0", name="gnd0")
                        nc.vector.tensor_sub(out=ysn, in0=ysb,
                                             in1=_bcast(st["mu_all"], 8 * ci, 8, HN))
                        nc.vector.tensor_mul(out=ysn, in0=ysn,
                                             in1=_bcast(st["rsg"], 8 * ci, 8, HN))
                        nc.gpsimd.tensor_mul(out=ysn, in0=ysn, in1=lnxt[:, 0:512])
                        nc.gpsimd.tensor_add(out=ysn, in0=ysn, in1=lnxt[:, 512:1024])
                        nc.vector.tensor_mul(out=ysn, in0=ysn, in1=st["g_tm"][ci])
                        for i in range(CT):
                            trp = psB.tile([128, L], BF16, tag="pw", name="pw")
                            nc.tensor.transpose(out=trp,
                                                in_=ysn[:, i * 128:(i + 1) * 128],
                                                identity=idh)
                            nc.scalar.activation(out=ztc[i][:, c0:c0 + L], in_=trp,
                                                 func=AF.Copy)
                    accs = [psA.tile([128, TTW], F32, tag=f"acc{m}",
                                     name=f"acc{m}") for m in range(4)]
                    wt = load_wblk(wbig[l][:, WB_O:WB_O + 512], [128, CT, 512])
                    for i in range(CT):
                        for m in range(4):
                            nc.tensor.matmul(out=accs[m],
                                             lhsT=wt[:, i, m * 128:(m + 1) * 128],
                                             rhs=ztc[i], start=(i == 0),
                                             stop=(i == CT - 1))
                    for m in range(4):
                        nc.vector.tensor_add(out=xb[m][:, sl],
                                             in0=src_res[m][:, sl],
                                             in1=accs[m])

                def cm_prep(tt, rows, rs):
                    xc = [kp.tile([128, TTW], BF16, tag=f"xt{i}", name=f"xt{i}") for i in range(CT)]
                    ln_apply(rows, rs, cm_srcs[tt], vecs,
                             V_LN2W, V_LN2B, xc)
                    xx2 = [kp.tile([128, TTW], BF16, tag=f"xx{i}", name=f"xx{i}") for i in range(CT)]
                    for i in range(CT):
                        nc.vector.tensor_sub(out=xx2[i][:, 1:TTW],
                                             in0=xc[i][:, 0:TTW - 1],
                                             in1=xc[i][:, 1:TTW])
                        nc.vector.scalar_tensor_tensor(
                            out=xx2[i][:, 0:1], in0=carry2[i], scalar=1.0,
                            in1=xc[i][:, 0:1], op0=OP.mult, op1=OP.subtract)
                        nc.gpsimd.tensor_copy(out=carry2[i], in_=xc[i][:, TTW - 1:TTW])
                    xk2 = [kp.tile([128, TTW], BF16, tag=f"xkh{tt}{i}", name=f"xkh{tt}{i}") for i in range(CT)]
                    xr2 = [kp.tile([128, TTW], BF16, tag=f"xrh{tt}{i}", name=f"xrh{tt}{i}") for i in range(CT)]
                    for i in range(CT):
                        xxk = k2.tile([128, TTW], BF16, tag="xxm", name="xxm", bufs=1)
                        nc.vector.tensor_scalar_mul(out=xxk, in0=xx2[i],
                                                    scalar1=vcol(vecs, i, V_CMK))
                        nc.vector.tensor_add(out=xk2[i], in0=xxk, in1=xc[i])
                        xxr = k2.tile([128, TTW], BF16, tag="xxm", name="xxm", bufs=1)
                        nc.vector.tensor_scalar_mul(out=xxr, in0=xx2[i],
                                                    scalar1=vcol(vecs, i, V_CMR))
                        nc.vector.tensor_add(out=xr2[i], in0=xxr, in1=xc[i])
                    return xk2, xr2

                # conv residue for the join gate (only needs xres)
                def cv_prep(tt):
                    sl = slice(tt * TTW, (tt + 1) * TTW)
                    cv = [kp.tile([128, TTW], BF16, tag=f"cvh{tt}{i}", name=f"cvh{tt}{i}") for i in range(CT)]
                    a = tt * TTW
                    for i in range(CT):
                        nc.scalar.activation(out=cv[i], in_=xres[i][:, sl].bitcast(F32),
                                             func=AF.Copy,
                                             scale=vcol(vecs, i, V_CW1))
                        lo = 1 if tt == 0 else 0
                        nc.vector.scalar_tensor_tensor(
                            out=cv[i][:, lo:TTW],
                            in0=xres[i][:, a + lo - 1:a + TTW - 1],
                            scalar=vcol(vecs, i, V_CW0),
                            in1=cv[i][:, lo:TTW], op0=OP.mult, op1=OP.add)
                        hi = TTW - 1 if tt == NTT - 1 else TTW
                        nc.vector.scalar_tensor_tensor(
                            out=cv[i][:, 0:hi],
                            in0=xres[i][:, a + 1:a + hi + 1],
                            scalar=vcol(vecs, i, V_CW2),
                            in1=cv[i][:, 0:hi], op0=OP.mult, op1=OP.add)
                    return cv

                def cm_wr_sig(tt, xr2):
                    # cm_Wr -> sigmoid(x) = 0.5*(1+tanh(x/2)); the 0.5 is
                    # folded into cm_Wv on the host
                    accs = [psA.tile([128, TTW], F32, tag=f"acc{m}", name=f"acc{m}") for m in range(4)]
                    wt = load_wblk(cmrg[l][:, 0:512], [128, CT, 512])
                    for i in range(CT):
                        for m in range(4):
                            nc.tensor.matmul(out=accs[m],
                                             lhsT=wt[:, i, m * 128:(m + 1) * 128],
                                             rhs=xr2[i], start=(i == 0),
                                             stop=(i == CT - 1))
                    sig = [kp.tile([128, TTW], BF16, tag=f"sig{m}", name=f"sig{m}") for m in range(4)]
                    for m in range(4):
                        e = k2.tile([128, TTW], BF16, tag="gte", name="gte", bufs=1)
                        tanh_act(accs[m], e, scale=0.5)
                        nc.vector.tensor_scalar_add(out=sig[m], in0=e, scalar1=1.0)
                    return sig

                class FfnEmitter:
                    # kk loop with cm_Wv accumulation; relu^2 on the ACT
                    # engine; emitted in slices so the serial WKV recurrence
                    # of the other frame can ride between them
                    def __init__(self, xk2):
                        self.xk2 = xk2
                        self.accs = [psA.tile([128, TTW], F32, tag=f"acc{m}",
                                              name=f"acc{m}") for m in range(4)]
                        self.f = 0
                        self.wfq = self.wvq = None

                    def emit(self, upto):
                        while self.f < min(upto, NFF):
                            f = self.f
                            fq, fr2 = f // 4, f % 4
                            nq = min(4, NFF - 4 * fq)
                            if fr2 == 0:
                                self.wfq = wp.tile([128, nq, CT, 128], BF16,
                                                   tag="wblk", name="wfq", bufs=2)
                                nc.sync.dma_start(
                                    out=self.wfq,
                                    in_=cmkp[l][4 * fq:4 * fq + nq]
                                    .rearrange("f (k p) n -> p f k n", p=128))
                                self.wvq = wp.tile([128, nq, C], BF16,
                                                   tag="wblk", name="wvq", bufs=2)
                                nc.sync.dma_start(
                                    out=self.wvq,
                                    in_=cmvp[l][4 * fq * 128:(4 * fq + nq) * 128, :]
                                    .rearrange("(f p) n -> p f n", p=128))
                            kkp = psB.tile([128, TTW], F32, tag="pw", name="pw")
                            for i in range(CT):
                                nc.tensor.matmul(out=kkp, lhsT=self.wfq[:, fr2, i, :],
                                                 rhs=self.xk2[i],
                                                 start=(i == 0), stop=(i == CT - 1))
                            kkf = k2.tile([128, TTW], BF16, tag="kkf", name="kkf")
                            nc.scalar.activation(out=kkf, in_=kkp, func=AF.Relu)
                            nc.scalar.activation(out=kkf, in_=kkf, func=AF.Square)
                            for m in range(4):
                                nc.tensor.matmul(out=self.accs[m],
                                                 lhsT=self.wvq[:, fr2, m * 128:(m + 1) * 128],
                                                 rhs=kkf, start=(f == 0),
                                                 stop=(f == NFF - 1))
                            self.f += 1

                def cm_tail(tt, sig, accs):
                    sl = slice(tt * TTW, (tt + 1) * TTW)
                    for m in range(4):
                        sg2 = k2.tile([128, TTW], F32, tag="lnt1", name="lnt1")
                        nc.vector.tensor_mul(out=sg2, in0=sig[m], in1=accs[m])
                        nc.gpsimd.tensor_add(out=xb[m][:, sl], in0=xb[m][:, sl],
                                             in1=sg2)
                    # half-frame exchange: reverse this half on-chip and gather
                    # it now; the time reversal maps our slot tt to the
                    # partner's slot 1-tt. Reversal stays on-chip because a
                    # reversed DRAM AP explodes into per-element descriptors.
                    sendh = dp.tile([C, TTW], BF16, tag=f"send{tt}",
                                    name=f"send{tt}")
                    recvh[tt] = dp.tile([2 * C, TTW], BF16, tag=f"recv{tt}",
                                        name=f"recv{tt}")
                    rv_keep = []
                    for i in range(CT):
                        rvt = kp.tile([128, TTW], BF16, tag="revT", name="revT",
                                      bufs=2)
                        nc.scalar.activation(
                            out=rvt, in_=_revap(xb[i][:, sl].bitcast(F32)),
                            func=AF.Copy)
                        nc.sync.dma_start(out=sendh[i * 128:(i + 1) * 128, :],
                                          in_=rvt)
                        rv_keep.append(rvt)
                    if solo:
                        for i in range(CT):
                            nc.sync.dma_start(
                                out=recvh[tt][i * 128:(i + 1) * 128, :],
                                in_=rv_keep[i])
                            nc.sync.dma_start(
                                out=recvh[tt][C + i * 128:C + (i + 1) * 128, :],
                                in_=rv_keep[i])
                    else:
                        nc.gpsimd.collective_compute(
                            "AllGather", OP.bypass, replica_groups=groups,
                            ins=[sendh.opt()], outs=[recvh[tt].opt()])

                # own/recv blend via tanh half-angle: t = tanh(0.5s(u+gbe));
                # out = mask05 * (own + recv + t*(own - recv))
                def join_gate(cv):
                    accs = [psA.tile([128, TTW], F32, tag=f"acc{m}", name=f"acc{m}") for m in range(4)]
                    wt = load_wblk(cmrg[l][:, 512:1024], [128, CT, 512])
                    for i in range(CT):
                        for m in range(4):
                            nc.tensor.matmul(out=accs[m],
                                             lhsT=wt[:, i, m * 128:(m + 1) * 128],
                                             rhs=cv[i], start=(i == 0),
                                             stop=(i == CT - 1))
                    es = []
                    for m in range(4):
                        e = k2.tile([128, TTW], BF16, tag="er", name="er",
                                    bufs=4)
                        nc.scalar.activation(out=e, in_=accs[m], func=AF.Tanh,
                                             scale=selt[:, S_NEGS:S_NEGS + 1],
                                             bias=vcol(vecs, m, V_GBM))
                        es.append(e)
                    return es

                def join_blend(tt, es):
                    sl = slice(tt * TTW, (tt + 1) * TTW)
                    recv = recvh[1 - tt]
                    for m in range(4):
                        jr0 = kp.tile([128, TTW], BF16, tag="jn0", name="jn0")
                        jr1 = kp.tile([128, TTW], BF16, tag="jn1", name="jn1")
                        nc.sync.dma_start(out=jr0, in_=recv[m * 128:(m + 1) * 128, :])
                        nc.sync.dma_start(out=jr1,
                                          in_=recv[C + m * 128:C + (m + 1) * 128, :])
                        # recv slot select (alpha,beta in {0,1})
                        nc.vector.tensor_scalar_mul(
                            out=jr0, in0=jr0, scalar1=selt[:, S_ALPHA:S_ALPHA + 1])
                        nc.vector.tensor_scalar_mul(
                            out=jr1, in0=jr1, scalar1=selt[:, S_BETA:S_BETA + 1])
                        jrs = kp.tile([128, TTW], BF16, tag="jn2", name="jn2")
                        nc.vector.tensor_add(out=jrs, in0=jr0, in1=jr1)
                        jsum = kp.tile([128, TTW], F32, tag="jn4", name="jn4")
                        nc.gpsimd.tensor_add(out=jsum, in0=xb[m][:, sl], in1=jrs)
                        d = kp.tile([128, TTW], BF16, tag="jn3", name="jn3")
                        nc.gpsimd.tensor_sub(out=d, in0=xb[m][:, sl], in1=jrs)
                        td = k2.tile([128, TTW], BF16, tag="jgd", name="jgd", bufs=1)
                        nc.vector.tensor_mul(out=td, in0=es[m], in1=d)
                        nc.gpsimd.tensor_add(out=jsum, in0=jsum, in1=td)
                        nc.vector.tensor_mul(out=xres[m][:, sl], in0=jsum,
                                             in1=maskt[:, sl])

                # pipeline: frame-1 precompute rides inside frame-0's serial
                # recurrence, frame-1's recurrence inside frame-0's FFN
                st0 = stage_a(0)
                for ci in range(NCPT):
                    stage_b(st0, ci)
                st1 = stage_a(1)
                for ci in range(NCPT):
                    stage_c(st0, ci)
                    stage_b(st1, ci)
                for ci in range(NCPT):
                    stage_dstats(st0, ci)
                stage_gn_ln(st0)
                stage_gn_exp(st0)
                stage_dnorm_wo(st0)

                cm_srcs = [[xb[i][:, tt * TTW:(tt + 1) * TTW] for i in range(CT)]
                           for tt in range(NTT)]
                recvh = [None, None]
                rows0, rs0 = ln_multi([cm_srcs[0]])
                xk0, xr0 = cm_prep(0, rows0[0], rs0[0])
                cv0 = cv_prep(0)
                sig0 = cm_wr_sig(0, xr0)
                ffn0 = FfnEmitter(xk0)
                for ci in range(NCPT):
                    stage_c(st1, ci)
                    ffn0.emit(3 * (ci + 1))
                ffn0.emit(NFF)
                cm_tail(0, sig0, ffn0.accs)
                for ci in range(NCPT):
                    stage_dstats(st1, ci)
                stage_gn_ln(st1)
                stage_gn_exp(st1)
                stage_dnorm_wo(st1)
                rows1, rs1 = ln_multi([cm_srcs[1]])
                xk1, xr1 = cm_prep(1, rows1[0], rs1[0])
                cv1 = cv_prep(1)
                sig1 = cm_wr_sig(1, xr1)
                ffn1 = FfnEmitter(xk1)
                ffn1.emit(NFF)
                es1 = join_gate(cv1)
                cm_tail(1, sig1, ffn1.accs)
                join_blend(1, es1)
                es0 = join_gate(cv0)
                join_blend(0, es0)
            # ---- output ----
            for i in range(CT):
                nc.sync.dma_start(out=xout[i * 128:(i + 1) * 128, :],
                                  in_=xres[i].bitcast(F32))
    nc.compile()
    return nc


def _host_inputs(inputs):
    import ml_dtypes
    bf16 = ml_dtypes.bfloat16
    x = np.asarray(inputs["x"], np.float32)
    lengths = np.asarray(inputs["lengths"]).astype(np.int64)
    pos = np.arange(T, dtype=np.float32)[:, None]
    div = np.exp(np.arange(0, C, 2, dtype=np.float32) * (-np.log(10000.0) / C))
    pe = np.zeros((T, C), np.float32)
    pe[:, 0::2] = np.sin(pos * div)
    pe[:, 1::2] = np.cos(pos * div)
    mask = (np.arange(T)[None, :] < lengths[:, None]).astype(np.float32)

    gw = np.asarray(inputs["gate_w"], np.float32)
    gb = np.asarray(inputs["gate_b"], np.float32)
    cw = np.asarray(inputs["conv_w"], np.float32)
    cb = np.asarray(inputs["conv_b"], np.float32)

    in_maps = []
    for c in range(8):
        b, d = c % 4, c // 4
        rev = d == 1
        s = -1.0 if rev else 1.0
        xin = (x[b] + pe)
        mrow = mask[b]
        if rev:
            xin = xin[::-1]
            mrow = mrow[::-1]
        # xb init: layer 0's first block applies ln0 on the forward branch
        # only; precompute it on the host
        if not rev:
            mu = xin.mean(-1, keepdims=True)
            var = ((xin - mu) ** 2).mean(-1, keepdims=True)
            x1 = ((xin - mu) / np.sqrt(var + 1e-5)
                  * np.asarray(inputs["ln0_w"], np.float32)
                  + np.asarray(inputs["ln0_b"], np.float32))
        else:
            x1 = xin
        m = {
            "x0": np.ascontiguousarray(xin.T),
            "x1": np.ascontiguousarray(x1.T.astype(np.float32)),
            # 0.5 fold: join uses the tanh half-angle form
            "mask05": np.ascontiguousarray(
                np.broadcast_to(0.5 * mrow, (128, T))).astype(bf16),
            "trib": np.triu(np.ones((128, 128), np.float32), 1).astype(bf16),
            "identh": np.eye(128, dtype=np.float32).astype(bf16),
        }
        sel = np.zeros((128, 8), np.float32)
        sel[:, S_LN0] = 0.0 if rev else 1.0
        sel[:, S_NEGS] = 0.5 * s
        sel[:, S_ALPHA] = 1.0 if rev else 0.0
        sel[:, S_BETA] = 0.0 if rev else 1.0
        m["sel"] = sel
        for l in range(NL):
            W = {k: np.asarray(inputs[k], np.float32)[d, l]
                 for k in ["ln1_w", "ln1_b", "ln2_w", "ln2_b", "maa_x", "maa_w",
                           "maa_k", "maa_v", "maa_r", "maa_g", "tm_w1", "tm_w2",
                           "td_w1", "td_w2", "time_decay", "Wr", "Wk", "Wv",
                           "Wg", "Wo", "lnx_w", "lnx_b", "cm_maa_k", "cm_maa_r",
                           "cm_Wk", "cm_Wv", "cm_Wr", "time_faaaa"]}
            m[f"wbig{l}"] = np.ascontiguousarray(np.concatenate(
                [W["Wr"], W["Wk"], W["Wv"], W["Wg"], W["tm_w1"], W["td_w1"],
                 W["Wo"]], axis=1)).astype(bf16)
            # tm_w2 with the matching maa vector folded in as an extra row
            maa_by_f = [W["maa_w"], W["maa_k"], W["maa_v"], W["maa_r"],
                        W["maa_g"]]
            w2e = np.zeros((5 * TME, C), np.float32)
            for f in range(5):
                w2e[f * TME:f * TME + TM] = W["tm_w2"][f]
                w2e[f * TME + TM] = maa_by_f[f]
            m[f"tmw2{l}"] = np.ascontiguousarray(w2e).astype(bf16)
            m[f"tdw2{l}"] = np.ascontiguousarray(W["td_w2"]).astype(bf16)
            m[f"cmk{l}"] = np.ascontiguousarray(
                W["cm_Wk"].reshape(C, NFF, 128).transpose(1, 0, 2)).astype(bf16)
            # 0.5 fold: cm sigmoid is computed as (1+tanh(x/2))
            m[f"cmv{l}"] = np.ascontiguousarray(0.5 * W["cm_Wv"]).astype(bf16)
            m[f"cmrg{l}"] = np.ascontiguousarray(
                np.concatenate([W["cm_Wr"], gw[l]], axis=1)).astype(bf16)
            cwe = cw[l] if not rev else cw[l][:, ::-1]
            gbe = cb[l] @ gw[l] + gb[l]
            vec = np.zeros((C, NV), np.float32)
            vec[:, V_LN1W] = W["ln1_w"]; vec[:, V_LN1B] = W["ln1_b"]
            vec[:, V_LN2W] = W["ln2_w"]; vec[:, V_LN2B] = W["ln2_b"]
            vec[:, V_MAAX] = W["maa_x"]; vec[:, V_MAAW] = W["maa_w"]
            vec[:, V_MAAK] = W["maa_k"]; vec[:, V_MAAV] = W["maa_v"]
            vec[:, V_MAAR] = W["maa_r"]; vec[:, V_MAAG] = W["maa_g"]
            vec[:, V_TDCY] = W["time_decay"]
            vec[:, V_CMK] = W["cm_maa_k"]; vec[:, V_CMR] = W["cm_maa_r"]
            vec[:, V_GBM] = 0.5 * s * gbe
            vec[:, V_CW0] = cwe[:, 0]
            vec[:, V_CW1] = cwe[:, 1] - 1.0
            vec[:, V_CW2] = cwe[:, 2]
            vec[:, V_LN0W] = np.asarray(inputs["ln0_w"], np.float32)
            vec[:, V_LN0B] = np.asarray(inputs["ln0_b"], np.float32)
            m[f"vecs{l}"] = vec
            # 0.5 fold: g is computed as x*(1+tanh(x/2)) = 2*silu(x)
            lnx = np.zeros((128, 1024), np.float32)
            lnx[:, 0:512] = 0.5 * W["lnx_w"][None, :]
            lnx[:, 512:1024] = 0.5 * W["lnx_b"][None, :]
            m[f"lnx{l}"] = lnx.astype(bf16)
            u = W["time_faaaa"].reshape(C)
            hmu = np.zeros((C, 8), np.float32)
            for h in range(H):
                hmu[h * HN:(h + 1) * HN, h] = u[h * HN:(h + 1) * HN]
            m[f"hmu{l}"] = hmu
        in_maps.append(m)
    return in_maps


def kernel(**inputs):
    if "nc" not in _CACHE:
        _CACHE["nc"] = _build(dbg=False)
    nc = _CACHE["nc"]
    in_maps = _host_inputs(inputs)
    res = run_bass_kernel_spmd(nc, in_maps, list(range(8)),
                               tmpdir=_CACHE.get("tmpdir"))
    _CACHE["last_results"] = res
    out = np.empty((B, T, C), np.float32)
    for b in range(B):
        out[b] = res.results[b]["xout"].T
    return out


if __name__ == "__main__":
    rng = np.random.default_rng(0)
    demo = None


# revision 19
# speedup vs baseline: 1.0552x; 1.0552x over previous
"""BiWKV6 encoder kernel for 8 Trainium2 NeuronCores.

Sharding: (batch, direction) -> 8 units, one per core; core c handles
batch c % 4, direction c // 4. Backward cores run the identical SPMD
program on time-reversed inputs; the only cross-core communication is a
pairwise AllGather of each layer's block output, written time-reversed
into the partner's domain. Within a core activations are channel-major
[C, T]; the WKV scan uses the chunked linear-attention formulation
(chunk 128) with log-space cumulative decay from the DVE prefix scan.

v4: software-pipelined across the two 512-token frames — the serial
WKV recurrence of frame 0 is chunk-interleaved with frame 1's
precompute, and frame 1's recurrence with frame 0's GroupNorm stats, so
the latency-bound S chain always has independent work beside it in the
in-order engine queues.  maa folded into the shift matmuls (ones-row),
bf16 token-shift pipeline (DVE 2x/4x fast modes), relu^2 on the ACT
engine, tanh-form gates (tanh shares the exp activation table, so the
only ACT table reloads are at the Ln clusters, which are paired across
frames), GroupNorm stats via segmented tensor_reduce with stride-0
broadcast normalization, premasked intra-chunk attention matrices.
"""
import numpy as np

import concourse.bass as bass
import concourse.tile as tile
from concourse import bacc, mybir
from concourse.bass_utils import run_bass_kernel_spmd

B, T, C = 4, 1024, 512
H, HN = 8, 64
L = 128
TTW = 512
NTT = T // TTW
NCPT = TTW // L
CT = C // 128
TM, TD, FFN, NL = 32, 64, 1792, 2
NFF = FFN // 128
EPS_LN, EPS_GN = 1e-5, 64e-5
NV = 19
TME = TM + 1  # tm_w2 rows + folded maa row

F32 = mybir.dt.float32
F32R = mybir.dt.float32r
BF16 = mybir.dt.bfloat16
AF = mybir.ActivationFunctionType
OP = mybir.AluOpType
AX = mybir.AxisListType

WB_R, WB_K, WB_V, WB_G, WB_TM1, WB_TD1, WB_O = 0, 512, 1024, 1536, 2048, 2208, 2272
WB_COLS = 2784
(V_LN1W, V_LN1B, V_LN2W, V_LN2B, V_MAAX, V_MAAW, V_MAAK, V_MAAV, V_MAAR,
 V_MAAG, V_TDCY, V_CMK, V_CMR, V_GBM, V_CW0, V_CW1, V_CW2, V_LN0W,
 V_LN0B) = range(NV)
S_LN0, S_NEGS, S_ALPHA, S_BETA = 0, 1, 2, 3

_CACHE = {}


def _revap(ap):
    n = ap.ap[-1][1]
    return bass.AP(tensor=ap.tensor, offset=ap.offset + (n - 1) * ap.ap[-1][0],
                   ap=[ap.ap[0], [-ap.ap[-1][0], n]])


def _bcast(t, off, n, rep):
    # [128, n] slice starting at free-offset `off`, each column repeated
    # `rep` times via a stride-0 inner AP dim
    return bass.AP(tensor=t.tensor, offset=t.offset + off,
                   ap=[t.ap[0], [1, n], [0, rep]])


def _build(dbg=False, solo=False):
    nc = bacc.Bacc("TRN2", target_bir_lowering=False, debug=False, num_devices=8)

    x0 = nc.declare_dram_parameter("x0", [C, T], F32, isOutput=False)
    x1 = nc.declare_dram_parameter("x1", [C, T], F32, isOutput=False)
    mask05 = nc.declare_dram_parameter("mask05", [128, T], BF16, isOutput=False)
    sel_in = nc.declare_dram_parameter("sel", [128, 8], F32, isOutput=False)
    trib_in = nc.declare_dram_parameter("trib", [128, 128], BF16, isOutput=False)
    wbig, tmw2, tdw2, cmkp, cmvp, cmrg, vecs_in, lnx_in, hmu_in = \
        [], [], [], [], [], [], [], [], []
    identh_in = nc.declare_dram_parameter("identh", [128, 128], BF16, isOutput=False)
    for l in range(NL):
        wbig.append(nc.declare_dram_parameter(f"wbig{l}", [C, WB_COLS], BF16, isOutput=False))
        tmw2.append(nc.declare_dram_parameter(f"tmw2{l}", [5 * TME, C], BF16, isOutput=False))
        tdw2.append(nc.declare_dram_parameter(f"tdw2{l}", [TD, C], BF16, isOutput=False))
        cmkp.append(nc.declare_dram_parameter(f"cmk{l}", [NFF, C, 128], BF16, isOutput=False))
        cmvp.append(nc.declare_dram_parameter(f"cmv{l}", [FFN, C], BF16, isOutput=False))
        cmrg.append(nc.declare_dram_parameter(f"cmrg{l}", [C, 1024], BF16, isOutput=False))
        vecs_in.append(nc.declare_dram_parameter(f"vecs{l}", [C, NV], F32, isOutput=False))
        lnx_in.append(nc.declare_dram_parameter(f"lnx{l}", [128, 1024], BF16, isOutput=False))
        hmu_in.append(nc.declare_dram_parameter(f"hmu{l}", [C, 8], F32, isOutput=False))
    xout = nc.declare_dram_parameter("xout", [C, T], F32, isOutput=True)

    groups = [[0, 4], [1, 5], [2, 6], [3, 7]]

    with tile.TileContext(nc) as tc:
        with (
            tc.tile_pool(name="pp", bufs=1) as pp,
            tc.tile_pool(name="wp", bufs=2) as wp,
            tc.tile_pool(name="kp", bufs=1) as kp,
            tc.tile_pool(name="k2", bufs=2) as k2,
            tc.tile_pool(name="psA", bufs=1, space="PSUM") as psA,
            tc.tile_pool(name="psB", bufs=2, space="PSUM") as psB,
            tc.tile_pool(name="psC", bufs=1, space="PSUM") as psC,
            tc.tile_pool(name="dp", bufs=2, space="DRAM") as dp,
        ):
            # ------------- persistent loads -------------
            xres = [pp.tile([128, T], F32R, tag=f"xres{i}", name=f"xres{i}") for i in range(CT)]
            xb = [pp.tile([128, T], F32R, tag=f"xb{i}", name=f"xb{i}") for i in range(CT)]
            for i in range(CT):
                nc.sync.dma_start(out=xres[i],
                                  in_=x0[i * 128:(i + 1) * 128, :].bitcast(F32R))
                nc.sync.dma_start(out=xb[i],
                                  in_=x1[i * 128:(i + 1) * 128, :].bitcast(F32R))
            maskt = pp.tile([128, T], BF16, tag="mask", name="mask")
            nc.sync.dma_start(out=maskt, in_=mask05[:, :])
            selt = pp.tile([128, 8], F32, tag="sel", name="sel")
            nc.sync.dma_start(out=selt, in_=sel_in[:, :])
            eps_ln_t = pp.tile([128, 1], F32, tag="epsln", name="epsln")
            nc.vector.memset(eps_ln_t, EPS_LN)
            eps_gn_t = pp.tile([128, 1], F32, tag="epsgn", name="epsgn")
            nc.vector.memset(eps_gn_t, EPS_GN)
            triu = pp.tile([128, 128], BF16, tag="triu", name="triu")
            nc.sync.dma_start(out=triu, in_=trib_in[:, :])
            idh = pp.tile([128, 128], BF16, tag="idh", name="idh")
            nc.sync.dma_start(out=idh, in_=identh_in[:, :])
            onesr_t = pp.tile([128, 128], F32R, tag="onesr", name="onesr")
            nc.vector.memset(onesr_t.bitcast(F32), 1.0)
            ones_r = onesr_t

            def vcol(vt, i, j):
                return vt[i][:, j:j + 1]

            def load_w(dram_ap, shape, tag, bufs=2):
                t = wp.tile(shape, BF16, tag=tag, name="wld", bufs=bufs)
                nc.sync.dma_start(out=t, in_=dram_ap)
                return t

            def load_wblk(dram_2d, shape):
                # one DMA for a [C, w] weight block -> SBUF [128, CT, w]
                t = wp.tile(shape, BF16, tag="wblk", name="wblk", bufs=2)
                nc.sync.dma_start(
                    out=t, in_=dram_2d.rearrange("(k p) n -> p k n", p=128))
                return t

            # LN in three phases so the Ln / Exp ops of several frames can be
            # emitted adjacently (each Ln->Exp family switch reloads the ACT
            # function table at ~1.3us)
            def ln_pre(src_sl):
                ssum = psC.tile([1, TTW], F32, tag="stA", name="stA")
                ssq = psC.tile([1, TTW], F32, tag="stB", name="stB")
                for i in range(CT):
                    sq = k2.tile([128, TTW], F32R, tag="lnsq", name="lnsq",
                                 bufs=1)
                    nc.gpsimd.tensor_mul(out=sq,
                                         in0=src_sl[i].bitcast(F32),
                                         in1=src_sl[i].bitcast(F32))
                    nc.tensor.matmul(out=ssum, lhsT=ones_r[:, 0:1], rhs=src_sl[i],
                                     start=(i == 0), stop=(i == CT - 1))
                    nc.tensor.matmul(out=ssq, lhsT=ones_r[:, 0:1], rhs=sq,
                                     start=(i == 0), stop=(i == CT - 1))
                rows = k2.tile([128, TTW], F32, tag="lnrows", name="lnrows",
                               bufs=2)
                srow, s2, varu = (rows[j:j + 1, :] for j in (0, 32, 64))
                nc.scalar.activation(out=srow, in_=ssum, func=AF.Copy)
                nc.vector.tensor_mul(out=s2, in0=srow, in1=srow)
                nc.vector.scalar_tensor_tensor(out=varu, in0=s2, scalar=-1.0 / C,
                                               in1=ssq, op0=OP.mult, op1=OP.add)
                return rows

            def ln_ln(rows):
                nc.scalar.activation(out=rows[96:97, :], in_=rows[64:65, :],
                                     func=AF.Ln, scale=1.0 / C,
                                     bias=eps_ln_t[0:1, :])

            def ln_exp(rows):
                rs = k2.tile([1, TTW], F32R, tag="lnrs", name="lnrs", bufs=2)
                nc.scalar.activation(out=rs, in_=rows[96:97, :], func=AF.Exp,
                                     scale=-0.5)
                return rs

            def ln_apply(rows, rs, src_sl, vt, wi, bi, out_tiles):
                murs = k2.tile([1, TTW], F32R, tag="lnmu", name="lnmu", bufs=1)
                nc.vector.scalar_tensor_tensor(out=murs, in0=rows[0:1, :],
                                               scalar=1.0 / C, in1=rs,
                                               op0=OP.mult, op1=OP.mult)
                bc0 = psB.tile([128, TTW], F32, tag="pw", name="pw")
                nc.tensor.matmul(out=bc0, lhsT=ones_r[0:1, 0:128], rhs=rs,
                                 start=True, stop=True)
                bc1 = psB.tile([128, TTW], F32, tag="pw", name="pw")
                nc.tensor.matmul(out=bc1, lhsT=ones_r[0:1, 0:128], rhs=murs,
                                 start=True, stop=True)
                for i in range(CT):
                    t1 = k2.tile([128, TTW], F32, tag="lnt1", name="lnt1")
                    nc.vector.tensor_mul(out=t1, in0=src_sl[i], in1=bc0)
                    nc.vector.tensor_sub(out=t1, in0=t1, in1=bc1)
                    nc.scalar.activation(out=out_tiles[i], in_=t1, func=AF.Identity,
                                         scale=vcol(vt, i, wi),
                                         bias=vcol(vt, i, bi))

            def ln_multi(srcs):
                rows_l = [ln_pre(s) for s in srcs]
                for r in rows_l:
                    ln_ln(r)
                rs_l = [ln_exp(r) for r in rows_l]
                return rows_l, rs_l

            def tanh_act(psum_ap, out_tile, scale=1.0):
                nc.scalar.activation(out=out_tile, in_=psum_ap, func=AF.Tanh,
                                     scale=scale)

            # ================= layers =================
            for l in range(NL):
                vecs = []
                for i in range(CT):
                    vt = pp.tile([128, NV], F32, tag=f"vecs{i}", name=f"vecs{i}")
                    nc.sync.dma_start(out=vt, in_=vecs_in[l][i * 128:(i + 1) * 128, :])
                    vecs.append(vt)
                lnxt = pp.tile([128, 1024], BF16, tag="lnx", name="lnx")
                nc.sync.dma_start(out=lnxt, in_=lnx_in[l][:, :])
                hmu = []
                for i in range(CT):
                    ht = pp.tile([128, 8], F32, tag=f"hmu{i}", name=f"hmu{i}")
                    nc.sync.dma_start(out=ht, in_=hmu_in[l][i * 128:(i + 1) * 128, :])
                    hmu.append(ht)

                # xb init for l==0 comes precomputed from the host (x1);
                # for l>0, xb is initialized lazily: time-mix LN reads xres
                # directly and the Wo accumulation writes xb = xres + Wo@y.

                S_box = [pp.tile([128, HN], BF16, tag=f"S{i}", name=f"S{i}") for i in range(CT)]
                for i in range(CT):
                    nc.vector.memset(S_box[i], 0.0)
                S_box = [S_box]  # boxed so stage_c can rebind
                carry = [pp.tile([128, 1], F32, tag=f"ca{i}", name=f"ca{i}") for i in range(CT)]
                carry2 = [pp.tile([128, 1], F32, tag=f"cb{i}", name=f"cb{i}") for i in range(CT)]
                for i in range(CT):
                    nc.gpsimd.memset(carry[i], 0.0)
                    nc.gpsimd.memset(carry2[i], 0.0)

                # ================= time mix =================
                src_res = xb if l == 0 else xres
                tm_srcs = [[src_res[i][:, tt * TTW:(tt + 1) * TTW]
                            for i in range(CT)] for tt in range(NTT)]
                tm_rows, tm_rs = ln_multi(tm_srcs)

                def stage_a(tt):
                    st = {"tt": tt, "sl": slice(tt * TTW, (tt + 1) * TTW)}
                    xt = [kp.tile([128, TTW], BF16, tag=f"xt{i}", name=f"xt{i}") for i in range(CT)]
                    ln_apply(tm_rows[tt], tm_rs[tt], tm_srcs[tt], vecs,
                             V_LN1W, V_LN1B, xt)
                    xx = [kp.tile([128, TTW], BF16, tag=f"xx{i}", name=f"xx{i}") for i in range(CT)]
                    for i in range(CT):
                        nc.vector.tensor_sub(out=xx[i][:, 1:TTW],
                                             in0=xt[i][:, 0:TTW - 1],
                                             in1=xt[i][:, 1:TTW])
                        nc.vector.scalar_tensor_tensor(
                            out=xx[i][:, 0:1], in0=carry[i], scalar=1.0,
                            in1=xt[i][:, 0:1], op0=OP.mult, op1=OP.subtract)
                        nc.gpsimd.tensor_copy(out=carry[i], in_=xt[i][:, TTW - 1:TTW])

                    # ---- t5 ----
                    mx = [k2.tile([128, TTW], BF16, tag=f"xf{i}", name=f"xf{i}") for i in range(CT)]
                    for i in range(CT):
                        xxm = k2.tile([128, TTW], BF16, tag="xxm", name="xxm", bufs=1)
                        nc.vector.tensor_scalar_mul(out=xxm, in0=xx[i],
                                                    scalar1=vcol(vecs, i, V_MAAX))
                        nc.vector.tensor_add(out=mx[i], in0=xxm, in1=xt[i])
                    p160a = psC.tile([128, TTW], F32, tag="stA", name="stA")
                    p160b = psC.tile([32, TTW], F32, tag="stB", name="stB")
                    wtmtd = wp.tile([128, CT, 224], BF16, tag="wtmtd", name="wtmtd",
                                    bufs=1)
                    nc.sync.dma_start(
                        out=wtmtd,
                        in_=wbig[l][:, WB_TM1:WB_TM1 + 224]
                        .rearrange("(k p) n -> p k n", p=128))
                    for i in range(CT):
                        nc.tensor.matmul(out=p160a, lhsT=wtmtd[:, i, 0:128], rhs=mx[i],
                                         start=(i == 0), stop=(i == CT - 1))
                        nc.tensor.matmul(out=p160b, lhsT=wtmtd[:, i, 128:160], rhs=mx[i],
                                         start=(i == 0), stop=(i == CT - 1))
                    # t5 tiles carry an extra all-ones row so the folded maa
                    # row of tmw2 lands as a bias in the shift matmuls
                    t5 = [k2.tile([TME, TTW], BF16, tag=f"t5{f}", name=f"t5{f}", bufs=1) for f in range(5)]
                    for f in range(5):
                        nc.vector.memset(t5[f][TM:TME, :], 1.0)
                    for f in range(4):
                        tanh_act(p160a[f * 32:(f + 1) * 32, :], t5[f][0:TM, :])
                    tanh_act(p160b, t5[4][0:TM, :])

                    def build_xf(fidx):
                        w2 = load_w(tmw2[l][fidx * TME:(fidx + 1) * TME, :],
                                    [TME, C], "wtm2")
                        xft = []
                        for i in range(CT):
                            dlp = psB.tile([128, TTW], F32, tag="pw", name="pw")
                            nc.tensor.matmul(out=dlp,
                                             lhsT=w2[:, i * 128:(i + 1) * 128],
                                             rhs=t5[fidx], start=True, stop=True)
                            a = k2.tile([128, TTW], BF16, tag="xfa", name="xfa")
                            nc.vector.tensor_mul(out=a, in0=dlp, in1=xx[i])
                            xf = k2.tile([128, TTW], BF16, tag=f"xf{i}", name=f"xf{i}")
                            nc.vector.tensor_add(out=xf, in0=a, in1=xt[i])
                            xft.append(xf)
                        return xft

                    def wmm(col_off, xft):
                        accs = [psA.tile([128, TTW], F32, tag=f"acc{m}",
                                         name=f"acc{m}") for m in range(4)]
                        wt = load_wblk(wbig[l][:, col_off:col_off + 512],
                                       [128, CT, 512])
                        for i in range(CT):
                            for m in range(4):
                                nc.tensor.matmul(out=accs[m],
                                                 lhsT=wt[:, i, m * 128:(m + 1) * 128],
                                                 rhs=xft[i], start=(i == 0),
                                                 stop=(i == CT - 1))
                        return accs

                    def wmm_tm(col_off, xft):
                        accs = [psA.tile([128, TTW], F32, tag=f"acc{m}",
                                         name=f"acc{m}") for m in range(4)]
                        wt = load_wblk(wbig[l][:, col_off:col_off + 512],
                                       [128, CT, 512])
                        for i in range(CT):
                            for ci in range(NCPT):
                                nc.tensor.matmul(out=accs[ci],
                                                 lhsT=xft[i][:, ci * L:(ci + 1) * L],
                                                 rhs=wt[:, i, :], start=(i == 0),
                                                 stop=(i == CT - 1))
                        return accs

                    # k
                    xf = build_xf(1)
                    accs = wmm(WB_K, xf)
                    k_sb = [kp.tile([128, TTW], BF16, tag=f"ksb{i}", name=f"ksb{i}") for i in range(CT)]
                    for m in range(4):
                        nc.scalar.activation(out=k_sb[m], in_=accs[m], func=AF.Copy)
                    # v token-major
                    xf = build_xf(2)
                    accs = wmm_tm(WB_V, xf)
                    v_tm = [kp.tile([128, C], BF16, tag=f"vtm{tt}{ci}", name=f"vtm{tt}{ci}") for ci in range(NCPT)]
                    for ci in range(NCPT):
                        nc.scalar.activation(out=v_tm[ci], in_=accs[ci], func=AF.Copy)
                    # r
                    xf = build_xf(3)
                    accs = wmm(WB_R, xf)
                    r_sb = [kp.tile([128, TTW], BF16, tag=f"rsb{i}", name=f"rsb{i}") for i in range(CT)]
                    for m in range(4):
                        nc.scalar.activation(out=r_sb[m], in_=accs[m], func=AF.Copy)
                    # g token-major: 2*silu(x) = x*(1+tanh(x/2)); the 0.5 is
                    # folded into lnx_w/lnx_b on the host
                    xf = build_xf(4)
                    accs = wmm_tm(WB_G, xf)
                    g_tm = [kp.tile([128, C], BF16, tag=f"gtm{tt}{ci}", name=f"gtm{tt}{ci}") for ci in range(NCPT)]
                    for ci in range(NCPT):
                        e = k2.tile([128, C], BF16, tag="gte", name="gte", bufs=1)
                        tanh_act(accs[ci], e, scale=0.5)
                        nc.vector.tensor_scalar_add(out=e, in0=e, scalar1=1.0)
                        nc.vector.tensor_mul(out=g_tm[ci], in0=e, in1=accs[ci])
                    # w -> wacc -> lai
                    xf = build_xf(0)
                    tdp = psC.tile([TD, TTW], F32, tag="stA", name="stA")
                    for i in range(CT):
                        nc.tensor.matmul(out=tdp, lhsT=wtmtd[:, i, 160:224], rhs=xf[i],
                                         start=(i == 0), stop=(i == CT - 1))
                    tdt = k2.tile([TD, TTW], BF16, tag="tdt", name="tdt", bufs=1)
                    tanh_act(tdp, tdt)
                    w2t = load_w(tdw2[l][:, :], [TD, C], "wtd2", bufs=1)
                    lai = [kp.tile([128, 1 + TTW], F32, tag=f"lai{i}", name=f"lai{i}") for i in range(CT)]
                    for i in range(CT):
                        wwp = psB.tile([128, TTW], F32, tag="pw", name="pw")
                        nc.tensor.matmul(out=wwp, lhsT=w2t[:, i * 128:(i + 1) * 128],
                                         rhs=tdt, start=True, stop=True)
                        wacc = k2.tile([128, TTW], F32, tag="lnt1", name="lnt1")
                        nc.scalar.activation(out=wacc, in_=wwp, func=AF.Exp,
                                             bias=vcol(vecs, i, V_TDCY))
                        nc.gpsimd.memset(lai[i][:, 0:1], 0.0)
                        nc.vector.tensor_tensor_scan(
                            out=lai[i][:, 1:1 + TTW], data0=wacc, data1=wacc,
                            initial=0.0, op0=OP.add, op1=OP.bypass)
                    st.update(k_sb=k_sb, r_sb=r_sb, v_tm=v_tm, g_tm=g_tm, lai=lai,
                              fn_a=[[None] * CT for _ in range(NCPT)],
                              rt_a=[[None] * CT for _ in range(NCPT)],
                              khtm_a=[[None] * CT for _ in range(NCPT)],
                              pts_a=[[[None] * 2 for _ in range(CT)]
                                     for _ in range(NCPT)],
                              dall_a=[None] * NCPT, ysb_a=[None] * NCPT)
                    return st

                # per-chunk precompute: everything that does not depend on the
                # serial S recurrence, incl. the premasked intra-chunk
                # attention matrices
                def stage_b(st, ci):
                    c0 = ci * L
                    lai, k_sb, r_sb = st["lai"], st["k_sb"], st["r_sb"]
                    mt_c = []
                    for i in range(CT):
                        ngc = k2.tile([128, 1], F32, tag="ngc", name="ngc", bufs=4)
                        nc.vector.tensor_scalar_mul(out=ngc,
                                                    in0=lai[i][:, c0:c0 + 1],
                                                    scalar1=-1.0)
                        fp = k2.tile([128, 1 + L], BF16, tag="fp", name="fp",
                                     bufs=2)
                        nc.scalar.activation(out=fp, in_=lai[i][:, c0:c0 + 1 + L],
                                             func=AF.Exp, bias=ngc)
                        fn = k2.tile([128, 1 + L], BF16, tag="fn", name="fn",
                                     bufs=2)
                        nc.scalar.activation(out=fn, in_=lai[i][:, c0:c0 + 1 + L],
                                             func=AF.Exp, scale=-1.0,
                                             bias=lai[i][:, c0:c0 + 1])
                        fnl = k2.tile([128, 1], F32, tag="fnl", name="fnl",
                                      bufs=NCPT * CT)
                        nc.gpsimd.tensor_copy(out=fnl, in_=fn[:, L:L + 1])
                        st["fn_a"][ci][i] = fnl
                        rt = k2.tile([128, L], BF16, tag="rt", name="rt",
                                     bufs=NCPT * CT)
                        nc.gpsimd.tensor_mul(out=rt, in0=r_sb[i][:, c0:c0 + L],
                                             in1=fn[:, 0:L])
                        kt = k2.tile([128, L], BF16, tag="kt", name="kt",
                                     bufs=2)
                        nc.gpsimd.tensor_mul(out=kt, in0=k_sb[i][:, c0:c0 + L],
                                             in1=fp[:, 1:1 + L])
                        kh = k2.tile([128, L], BF16, tag="kh", name="kh", bufs=2)
                        nc.vector.tensor_scalar_mul(out=kh, in0=kt, scalar1=fnl)
                        mt = k2.tile([128, L], F32, tag="mt", name="mt", bufs=4)
                        nc.gpsimd.tensor_mul(out=mt, in0=r_sb[i][:, c0:c0 + L],
                                             in1=k_sb[i][:, c0:c0 + L])
                        mt_c.append(mt)
                        st["rt_a"][ci][i] = rt
                        trp = psB.tile([128, L], BF16, tag="pw", name="pw")
                        nc.tensor.transpose(out=trp, in_=kh, identity=idh)
                        kht = k2.tile([128, L], BF16, tag="khtm", name="khtm",
                                      bufs=NCPT * CT)
                        nc.scalar.activation(out=kht, in_=trp, func=AF.Copy)
                        st["khtm_a"][ci][i] = kht
                        for hh in range(2):
                            hb = hh * HN
                            pt = psB.tile([L, L], F32, tag="pw", name="pw")
                            nc.tensor.matmul(out=pt, lhsT=kt[hb:hb + HN, :],
                                             rhs=rt[hb:hb + HN, :],
                                             start=True, stop=True)
                            pts = k2.tile([L, L], BF16, tag="pts", name="pts",
                                          bufs=2 * NCPT * CT)
                            nc.vector.tensor_mul(out=pts, in0=pt, in1=triu)
                            st["pts_a"][ci][i][hh] = pts
                    dall = psC.tile([128, 8], F32, tag="stB", name="stB")
                    for i in range(CT):
                        nc.tensor.matmul(out=dall, lhsT=mt_c[i], rhs=hmu[i],
                                         start=(i == 0), stop=(i == CT - 1))
                    dsb = k2.tile([128, 8], F32, tag="dsb", name="dsb",
                                  bufs=NCPT)
                    nc.scalar.activation(out=dsb, in_=dall, func=AF.Copy)
                    st["dall_a"][ci] = dsb

                # one chunk of the serial S recurrence (gn deferred)
                def stage_c(st, ci):
                    tt = st["tt"]
                    gc = tt * NCPT + ci
                    rt_t, kh_tm = st["rt_a"][ci], st["khtm_a"][ci]
                    v_tm = st["v_tm"]
                    S_cur = S_box[0]
                    yps = psA.tile([128, C], F32, tag=f"acc{ci % 2}", name="yps")
                    S_new = [k2.tile([128, HN], BF16, tag=f"Sn{i}", name=f"Sn{i}") for i in range(CT)]
                    for i in range(CT):
                        sup = psC.tile([128, HN], F32,
                                       tag=("stA" if i % 2 == 0 else "stB"),
                                       name="sup")
                        for hh in range(2):
                            h = 2 * i + hh
                            hb = hh * HN
                            nc.tensor.matmul(
                                out=yps[:, h * HN:(h + 1) * HN],
                                lhsT=st["pts_a"][ci][i][hh],
                                rhs=v_tm[ci][:, h * HN:(h + 1) * HN],
                                start=True, stop=(gc == 0), skip_group_check=True)
                            if gc > 0:
                                nc.tensor.matmul(
                                    out=yps[:, h * HN:(h + 1) * HN],
                                    lhsT=rt_t[i][hb:hb + HN, :],
                                    rhs=S_cur[i][hb:hb + HN, :],
                                    start=False, stop=True, skip_group_check=True)
                            nc.tensor.matmul(
                                out=sup[hb:hb + HN, :],
                                lhsT=kh_tm[i][:, hb:hb + HN],
                                rhs=v_tm[ci][:, h * HN:(h + 1) * HN],
                                start=True, stop=True, skip_group_check=True)
                        nc.vector.scalar_tensor_tensor(
                            out=S_new[i], in0=S_cur[i], scalar=st["fn_a"][ci][i],
                            in1=sup, op0=OP.mult, op1=OP.add)
                    S_box[0] = S_new
                    # u-term: ysb = v * dall_bcast + yps
                    tmpv = k2.tile([128, C], BF16, tag="ytmp", name="ytmp", bufs=1)
                    nc.vector.tensor_mul(out=tmpv, in0=v_tm[ci],
                                         in1=_bcast(st["dall_a"][ci], 0, 8, HN))
                    ysb = k2.tile([128, C], BF16, tag=f"ysb{tt}{ci}",
                                  name=f"ysb{tt}{ci}", bufs=1)
                    nc.vector.tensor_add(out=ysb, in0=tmpv, in1=yps)
                    st["ysb_a"][ci] = ysb

                # groupnorm stats for one chunk (segmented tensor_reduce)
                def stage_dstats(st, ci):
                    tt = st["tt"]
                    if ci == 0:
                        st["mu_all"] = k2.tile([128, 8 * NCPT], F32,
                                               tag=f"gnmu{tt}", name=f"gnmu{tt}")
                        st["var_all"] = k2.tile([128, 8 * NCPT], F32,
                                                tag=f"gnvar{tt}", name=f"gnvar{tt}")
                    ysb = st["ysb_a"][ci]
                    sqt = k2.tile([128, C], BF16, tag="gnsq", name="gnsq", bufs=1)
                    nc.scalar.activation(out=sqt, in_=ysb, func=AF.Square)
                    suv = k2.tile([128, 8], F32, tag="gnsu", name="gnsu")
                    yv = bass.AP(tensor=ysb.tensor, offset=ysb.offset,
                                 ap=[ysb.ap[0], [HN, 8], [1, HN]])
                    nc.vector.tensor_reduce(out=suv, in_=yv, axis=AX.X, op=OP.add)
                    sqv = k2.tile([128, 8], F32, tag="gnsv", name="gnsv")
                    qv = bass.AP(tensor=sqt.tensor, offset=sqt.offset,
                                 ap=[sqt.ap[0], [HN, 8], [1, HN]])
                    nc.vector.tensor_reduce(out=sqv, in_=qv, axis=AX.X, op=OP.add)
                    mu = st["mu_all"][:, 8 * ci:8 * ci + 8]
                    nc.vector.tensor_scalar_mul(out=mu, in0=suv, scalar1=1.0 / HN)
                    msq = k2.tile([128, 8], F32, tag="gnms", name="gnms")
                    nc.vector.tensor_mul(out=msq, in0=mu, in1=mu)
                    nc.vector.scalar_tensor_tensor(
                        out=st["var_all"][:, 8 * ci:8 * ci + 8], in0=sqv,
                        scalar=1.0 / HN, in1=msq, op0=OP.mult, op1=OP.subtract)

                def stage_gn_ln(st):
                    lnv = k2.tile([128, 8 * NCPT], F32, tag="gnln", name="gnln")
                    nc.scalar.activation(out=lnv, in_=st["var_all"][:, :],
                                         func=AF.Ln, bias=eps_gn_t)
                    st["lnv"] = lnv

                def stage_gn_exp(st):
                    rsg = k2.tile([128, 8 * NCPT], BF16, tag="gnrs", name="gnrs")
                    nc.scalar.activation(out=rsg, in_=st["lnv"], func=AF.Exp,
                                         scale=-0.5)
                    st["rsg"] = rsg

                # normalize + affine + *g + transpose into ztc, then Wo
                def stage_dnorm_wo(st):
                    tt = st["tt"]
                    sl = st["sl"]
                    ztc = [kp.tile([128, TTW], BF16, tag=f"ztc{i}", name=f"ztc{i}") for i in range(CT)]
                    for ci in range(NCPT):
                        c0 = ci * L
                        ysb = st["ysb_a"][ci]
                        ysn = k2.tile([128, C], BF16, tag="gnd0", name="gnd0")
                        nc.vector.tensor_sub(out=ysn, in0=ysb,
                                             in1=_bcast(st["mu_all"], 8 * ci, 8, HN))
                        nc.vector.tensor_mul(out=ysn, in0=ysn,
                                             in1=_bcast(st["rsg"], 8 * ci, 8, HN))
                        nc.gpsimd.tensor_mul(out=ysn, in0=ysn, in1=lnxt[:, 0:512])
                        nc.gpsimd.tensor_add(out=ysn, in0=ysn, in1=lnxt[:, 512:1024])
                        nc.vector.tensor_mul(out=ysn, in0=ysn, in1=st["g_tm"][ci])
                        for i in range(CT):
                            trp = psB.tile([128, L], BF16, tag="pw", name="pw")
                            nc.tensor.transpose(out=trp,
                                                in_=ysn[:, i * 128:(i + 1) * 128],
                                                identity=idh)
                            nc.scalar.activation(out=ztc[i][:, c0:c0 + L], in_=trp,
                                                 func=AF.Copy)
                    accs = [psA.tile([128, TTW], F32, tag=f"acc{m}",
                                     name=f"acc{m}") for m in range(4)]
                    wt = load_wblk(wbig[l][:, WB_O:WB_O + 512], [128, CT, 512])
                    for i in range(CT):
                        for m in range(4):
                            nc.tensor.matmul(out=accs[m],
                                             lhsT=wt[:, i, m * 128:(m + 1) * 128],
                                             rhs=ztc[i], start=(i == 0),
                                             stop=(i == CT - 1))
                    for m in range(4):
                        nc.vector.tensor_add(out=xb[m][:, sl],
                                             in0=src_res[m][:, sl],
                                             in1=accs[m])

                def cm_prep(tt, rows, rs):
                    xc = [kp.tile([128, TTW], BF16, tag=f"xt{i}", name=f"xt{i}") for i in range(CT)]
                    ln_apply(rows, rs, cm_srcs[tt], vecs,
                             V_LN2W, V_LN2B, xc)
                    xx2 = [kp.tile([128, TTW], BF16, tag=f"xx{i}", name=f"xx{i}") for i in range(CT)]
                    for i in range(CT):
                        nc.vector.tensor_sub(out=xx2[i][:, 1:TTW],
                                             in0=xc[i][:, 0:TTW - 1],
                                             in1=xc[i][:, 1:TTW])
                        nc.vector.scalar_tensor_tensor(
                            out=xx2[i][:, 0:1], in0=carry2[i], scalar=1.0,
                            in1=xc[i][:, 0:1], op0=OP.mult, op1=OP.subtract)
                        nc.gpsimd.tensor_copy(out=carry2[i], in_=xc[i][:, TTW - 1:TTW])
                    xk2 = [kp.tile([128, TTW], BF16, tag=f"xkh{tt}{i}", name=f"xkh{tt}{i}") for i in range(CT)]
                    xr2 = [kp.tile([128, TTW], BF16, tag=f"xrh{tt}{i}", name=f"xrh{tt}{i}") for i in range(CT)]
                    for i in range(CT):
                        xxk = k2.tile([128, TTW], BF16, tag="xxm", name="xxm", bufs=1)
                        nc.vector.tensor_scalar_mul(out=xxk, in0=xx2[i],
                                                    scalar1=vcol(vecs, i, V_CMK))
                        nc.vector.tensor_add(out=xk2[i], in0=xxk, in1=xc[i])
                        xxr = k2.tile([128, TTW], BF16, tag="xxm", name="xxm", bufs=1)
                        nc.vector.tensor_scalar_mul(out=xxr, in0=xx2[i],
                                                    scalar1=vcol(vecs, i, V_CMR))
                        nc.vector.tensor_add(out=xr2[i], in0=xxr, in1=xc[i])
                    return xk2, xr2

                # conv residue for the join gate (only needs xres)
                def cv_prep(tt):
                    sl = slice(tt * TTW, (tt + 1) * TTW)
                    cv = [kp.tile([128, TTW], BF16, tag=f"cvh{tt}{i}", name=f"cvh{tt}{i}") for i in range(CT)]
                    a = tt * TTW
                    for i in range(CT):
                        nc.scalar.activation(out=cv[i], in_=xres[i][:, sl].bitcast(F32),
                                             func=AF.Copy,
                                             scale=vcol(vecs, i, V_CW1))
                        lo = 1 if tt == 0 else 0
                        nc.vector.scalar_tensor_tensor(
                            out=cv[i][:, lo:TTW],
                            in0=xres[i][:, a + lo - 1:a + TTW - 1],
                            scalar=vcol(vecs, i, V_CW0),
                            in1=cv[i][:, lo:TTW], op0=OP.mult, op1=OP.add)
                        hi = TTW - 1 if tt == NTT - 1 else TTW
                        nc.vector.scalar_tensor_tensor(
                            out=cv[i][:, 0:hi],
                            in0=xres[i][:, a + 1:a + hi + 1],
                            scalar=vcol(vecs, i, V_CW2),
                            in1=cv[i][:, 0:hi], op0=OP.mult, op1=OP.add)
                    return cv

                def cm_wr_sig(tt, xr2):
                    # cm_Wr -> sigmoid(x) = 0.5*(1+tanh(x/2)); the 0.5 is
                    # folded into cm_Wv on the host
                    accs = [psA.tile([128, TTW], F32, tag=f"acc{m}", name=f"acc{m}") for m in range(4)]
                    wt = load_wblk(cmrg[l][:, 0:512], [128, CT, 512])
                    for i in range(CT):
                        for m in range(4):
                            nc.tensor.matmul(out=accs[m],
                                             lhsT=wt[:, i, m * 128:(m + 1) * 128],
                                             rhs=xr2[i], start=(i == 0),
                                             stop=(i == CT - 1))
                    sig = [kp.tile([128, TTW], BF16, tag=f"sig{m}", name=f"sig{m}") for m in range(4)]
                    for m in range(4):
                        e = k2.tile([128, TTW], BF16, tag="gte", name="gte", bufs=1)
                        tanh_act(accs[m], e, scale=0.5)
                        nc.vector.tensor_scalar_add(out=sig[m], in0=e, scalar1=1.0)
                    return sig

                class FfnEmitter:
                    # kk loop with cm_Wv accumulation; relu^2 on the ACT
                    # engine; emitted in slices so the serial WKV recurrence
                    # of the other frame can ride between them
                    def __init__(self, xk2):
                        self.xk2 = xk2
                        self.accs = [psA.tile([128, TTW], F32, tag=f"acc{m}",
                                              name=f"acc{m}") for m in range(4)]
                        self.f = 0
                        self.wfq = self.wvq = None

                    def emit(self, upto):
                        while self.f < min(upto, NFF):
                            f = self.f
                            fq, fr2 = f // 4, f % 4
                            nq = min(4, NFF - 4 * fq)
                            if fr2 == 0:
                                self.wfq = wp.tile([128, nq, CT, 128], BF16,
                                                   tag="wblk", name="wfq", bufs=2)
                                nc.sync.dma_start(
                                    out=self.wfq,
                                    in_=cmkp[l][4 * fq:4 * fq + nq]
                                    .rearrange("f (k p) n -> p f k n", p=128))
                                self.wvq = wp.tile([128, nq, C], BF16,
                                                   tag="wblk", name="wvq", bufs=2)
                                nc.sync.dma_start(
                                    out=self.wvq,
                                    in_=cmvp[l][4 * fq * 128:(4 * fq + nq) * 128, :]
                                    .rearrange("(f p) n -> p f n", p=128))
                            kkp = psB.tile([128, TTW], F32, tag="pw", name="pw")
                            for i in range(CT):
                                nc.tensor.matmul(out=kkp, lhsT=self.wfq[:, fr2, i, :],
                                                 rhs=self.xk2[i],
                                                 start=(i == 0), stop=(i == CT - 1))
                            kkf = k2.tile([128, TTW], BF16, tag="kkf", name="kkf")
                            nc.scalar.activation(out=kkf, in_=kkp, func=AF.Relu)
                            nc.scalar.activation(out=kkf, in_=kkf, func=AF.Square)
                            for m in range(4):
                                nc.tensor.matmul(out=self.accs[m],
                                                 lhsT=self.wvq[:, fr2, m * 128:(m + 1) * 128],
                                                 rhs=kkf, start=(f == 0),
                                                 stop=(f == NFF - 1))
                            self.f += 1

                def cm_tail(tt, sig, accs):
                    sl = slice(tt * TTW, (tt + 1) * TTW)
                    for m in range(4):
                        sg2 = k2.tile([128, TTW], F32, tag="lnt1", name="lnt1")
                        nc.vector.tensor_mul(out=sg2, in0=sig[m], in1=accs[m])
                        nc.gpsimd.tensor_add(out=xb[m][:, sl], in0=xb[m][:, sl],
                                             in1=sg2)
                    # half-frame exchange: reverse this half on-chip and gather
                    # it now; the time reversal maps our slot tt to the
                    # partner's slot 1-tt. Reversal stays on-chip because a
                    # reversed DRAM AP explodes into per-element descriptors.
                    sendh = dp.tile([C, TTW], BF16, tag=f"send{tt}",
                                    name=f"send{tt}")
                    recvh[tt] = dp.tile([2 * C, TTW], BF16, tag=f"recv{tt}",
                                        name=f"recv{tt}")
                    rv_keep = []
                    for i in range(CT):
                        rvt = kp.tile([128, TTW], BF16, tag="revT", name="revT",
                                      bufs=2)
                        nc.scalar.activation(
                            out=rvt, in_=_revap(xb[i][:, sl].bitcast(F32)),
                            func=AF.Copy)
                        nc.sync.dma_start(out=sendh[i * 128:(i + 1) * 128, :],
                                          in_=rvt)
                        rv_keep.append(rvt)
                    if solo:
                        for i in range(CT):
                            nc.sync.dma_start(
                                out=recvh[tt][i * 128:(i + 1) * 128, :],
                                in_=rv_keep[i])
                            nc.sync.dma_start(
                                out=recvh[tt][C + i * 128:C + (i + 1) * 128, :],
                                in_=rv_keep[i])
                    else:
                        nc.gpsimd.collective_compute(
                            "AllGather", OP.bypass, replica_groups=groups,
                            ins=[sendh.opt()], outs=[recvh[tt].opt()])

                # own/recv blend via tanh half-angle: t = tanh(0.5s(u+gbe));
                # out = mask05 * (own + recv + t*(own - recv))
                def join_gate(cv):
                    accs = [psA.tile([128, TTW], F32, tag=f"acc{m}", name=f"acc{m}") for m in range(4)]
                    wt = load_wblk(cmrg[l][:, 512:1024], [128, CT, 512])
                    for i in range(CT):
                        for m in range(4):
                            nc.tensor.matmul(out=accs[m],
                                             lhsT=wt[:, i, m * 128:(m + 1) * 128],
                                             rhs=cv[i], start=(i == 0),
                                             stop=(i == CT - 1))
                    es = []
                    for m in range(4):
                        e = k2.tile([128, TTW], BF16, tag="er", name="er",
                                    bufs=4)
                        nc.scalar.activation(out=e, in_=accs[m], func=AF.Tanh,
                                             scale=selt[:, S_NEGS:S_NEGS + 1],
                                             bias=vcol(vecs, m, V_GBM))
                        es.append(e)
                    return es

                def join_blend(tt, es):
                    sl = slice(tt * TTW, (tt + 1) * TTW)
                    recv = recvh[1 - tt]
                    for m in range(4):
                        jr0 = kp.tile([128, TTW], BF16, tag="jn0", name="jn0")
                        jr1 = kp.tile([128, TTW], BF16, tag="jn1", name="jn1")
                        nc.sync.dma_start(out=jr0, in_=recv[m * 128:(m + 1) * 128, :])
                        nc.sync.dma_start(out=jr1,
                                          in_=recv[C + m * 128:C + (m + 1) * 128, :])
                        # recv slot select (alpha,beta in {0,1})
                        nc.vector.tensor_scalar_mul(
                            out=jr0, in0=jr0, scalar1=selt[:, S_ALPHA:S_ALPHA + 1])
                        nc.vector.tensor_scalar_mul(
                            out=jr1, in0=jr1, scalar1=selt[:, S_BETA:S_BETA + 1])
                        jrs = kp.tile([128, TTW], BF16, tag="jn2", name="jn2")
                        nc.vector.tensor_add(out=jrs, in0=jr0, in1=jr1)
                        jsum = kp.tile([128, TTW], F32, tag="jn4", name="jn4")
                        nc.gpsimd.tensor_add(out=jsum, in0=xb[m][:, sl], in1=jrs)
                        d = kp.tile([128, TTW], BF16, tag="jn3", name="jn3")
                        nc.gpsimd.tensor_sub(out=d, in0=xb[m][:, sl], in1=jrs)
                        td = k2.tile([128, TTW], BF16, tag="jgd", name="jgd", bufs=1)
                        nc.vector.tensor_mul(out=td, in0=es[m], in1=d)
                        nc.gpsimd.tensor_add(out=jsum, in0=jsum, in1=td)
                        nc.vector.tensor_mul(out=xres[m][:, sl], in0=jsum,
                                             in1=maskt[:, sl])

                # pipeline: frame-1 precompute rides inside frame-0's serial
                # recurrence, frame-1's recurrence inside frame-0's FFN
                st0 = stage_a(0)
                for ci in range(NCPT):
                    stage_b(st0, ci)
                st1 = stage_a(1)
                for ci in range(NCPT):
                    stage_c(st0, ci)
                    stage_b(st1, ci)
                for ci in range(NCPT):
                    stage_c(st1, ci)
                    stage_dstats(st0, ci)
                for ci in range(NCPT):
                    stage_dstats(st1, ci)
                stage_gn_ln(st0)
                stage_gn_ln(st1)
                stage_gn_exp(st0)
                stage_gn_exp(st1)
                stage_dnorm_wo(st0)
                stage_dnorm_wo(st1)

                cm_srcs = [[xb[i][:, tt * TTW:(tt + 1) * TTW] for i in range(CT)]
                           for tt in range(NTT)]
                recvh = [None, None]
                rows01, rs01 = ln_multi(cm_srcs)
                xk0, xr0 = cm_prep(0, rows01[0], rs01[0])
                xk1, xr1 = cm_prep(1, rows01[1], rs01[1])
                cv0 = cv_prep(0)
                cv1 = cv_prep(1)
                sig0 = cm_wr_sig(0, xr0)
                ffn0 = FfnEmitter(xk0)
                ffn0.emit(NFF)
                cm_tail(0, sig0, ffn0.accs)
                es1 = join_gate(cv1)
                sig1 = cm_wr_sig(1, xr1)
                ffn1 = FfnEmitter(xk1)
                ffn1.emit(NFF)
                cm_tail(1, sig1, ffn1.accs)
                join_blend(1, es1)
                es0 = join_gate(cv0)
                join_blend(0, es0)
            # ---- output ----
            for i in range(CT):
                nc.sync.dma_start(out=xout[i * 128:(i + 1) * 128, :],
                                  in_=xres[i].bitcast(F32))
    nc.compile()
    return nc


def _host_inputs(inputs):
    import ml_dtypes
    bf16 = ml_dtypes.bfloat16
    x = np.asarray(inputs["x"], np.float32)
    lengths = np.asarray(inputs["lengths"]).astype(np.int64)
    pos = np.arange(T, dtype=np.float32)[:, None]
    div = np.exp(np.arange(0, C, 2, dtype=np.float32) * (-np.log(10000.0) / C))
    pe = np.zeros((T, C), np.float32)
    pe[:, 0::2] = np.sin(pos * div)
    pe[:, 1::2] = np.cos(pos * div)
    mask = (np.arange(T)[None, :] < lengths[:, None]).astype(np.float32)

    gw = np.asarray(inputs["gate_w"], np.float32)
    gb = np.asarray(inputs["gate_b"], np.float32)
    cw = np.asarray(inputs["conv_w"], np.float32)
    cb = np.asarray(inputs["conv_b"], np.float32)

    in_maps = []
    for c in range(8):
        b, d = c % 4, c // 4
        rev = d == 1
        s = -1.0 if rev else 1.0
        xin = (x[b] + pe)
        mrow = mask[b]
        if rev:
            xin = xin[::-1]
            mrow = mrow[::-1]
        # xb init: layer 0's first block applies ln0 on the forward branch
        # only; precompute it on the host
        if not rev:
            mu = xin.mean(-1, keepdims=True)
            var = ((xin - mu) ** 2).mean(-1, keepdims=True)
            x1 = ((xin - mu) / np.sqrt(var + 1e-5)
                  * np.asarray(inputs["ln0_w"], np.float32)
                  + np.asarray(inputs["ln0_b"], np.float32))
        else:
            x1 = xin
        m = {
            "x0": np.ascontiguousarray(xin.T),
            "x1": np.ascontiguousarray(x1.T.astype(np.float32)),
            # 0.5 fold: join uses the tanh half-angle form
            "mask05": np.ascontiguousarray(
                np.broadcast_to(0.5 * mrow, (128, T))).astype(bf16),
            "trib": np.triu(np.ones((128, 128), np.float32), 1).astype(bf16),
            "identh": np.eye(128, dtype=np.float32).astype(bf16),
        }
        sel = np.zeros((128, 8), np.float32)
        sel[:, S_LN0] = 0.0 if rev else 1.0
        sel[:, S_NEGS] = 0.5 * s
        sel[:, S_ALPHA] = 1.0 if rev else 0.0
        sel[:, S_BETA] = 0.0 if rev else 1.0
        m["sel"] = sel
        for l in range(NL):
            W = {k: np.asarray(inputs[k], np.float32)[d, l]
                 for k in ["ln1_w", "ln1_b", "ln2_w", "ln2_b", "maa_x", "maa_w",
                           "maa_k", "maa_v", "maa_r", "maa_g", "tm_w1", "tm_w2",
                           "td_w1", "td_w2", "time_decay", "Wr", "Wk", "Wv",
                           "Wg", "Wo", "lnx_w", "lnx_b", "cm_maa_k", "cm_maa_r",
                           "cm_Wk", "cm_Wv", "cm_Wr", "time_faaaa"]}
            m[f"wbig{l}"] = np.ascontiguousarray(np.concatenate(
                [W["Wr"], W["Wk"], W["Wv"], W["Wg"], W["tm_w1"], W["td_w1"],
                 W["Wo"]], axis=1)).astype(bf16)
            # tm_w2 with the matching maa vector folded in as an extra row
            maa_by_f = [W["maa_w"], W["maa_k"], W["maa_v"], W["maa_r"],
                        W["maa_g"]]
            w2e = np.zeros((5 * TME, C), np.float32)
            for f in range(5):
                w2e[f * TME:f * TME + TM] = W["tm_w2"][f]
                w2e[f * TME + TM] = maa_by_f[f]
            m[f"tmw2{l}"] = np.ascontiguousarray(w2e).astype(bf16)
            m[f"tdw2{l}"] = np.ascontiguousarray(W["td_w2"]).astype(bf16)
            m[f"cmk{l}"] = np.ascontiguousarray(
                W["cm_Wk"].reshape(C, NFF, 128).transpose(1, 0, 2)).astype(bf16)
            # 0.5 fold: cm sigmoid is computed as (1+tanh(x/2))
            m[f"cmv{l}"] = np.ascontiguousarray(0.5 * W["cm_Wv"]).astype(bf16)
            m[f"cmrg{l}"] = np.ascontiguousarray(
                np.concatenate([W["cm_Wr"], gw[l]], axis=1)).astype(bf16)
            cwe = cw[l] if not rev else cw[l][:, ::-1]
            gbe = cb[l] @ gw[l] + gb[l]
            vec = np.zeros((C, NV), np.float32)
            vec[:, V_LN1W] = W["ln1_w"]; vec[:, V_LN1B] = W["ln1_b"]
            vec[:, V_LN2W] = W["ln2_w"]; vec[:, V_LN2B] = W["ln2_b"]
            vec[:, V_MAAX] = W["maa_x"]; vec[:, V_MAAW] = W["maa_w"]
            vec[:, V_MAAK] = W["maa_k"]; vec[:, V_MAAV] = W["maa_v"]
            vec[:, V_MAAR] = W["maa_r"]; vec[:, V_MAAG] = W["maa_g"]
            vec[:, V_TDCY] = W["time_decay"]
            vec[:, V_CMK] = W["cm_maa_k"]; vec[:, V_CMR] = W["cm_maa_r"]
            vec[:, V_GBM] = 0.5 * s * gbe
            vec[:, V_CW0] = cwe[:, 0]
            vec[:, V_CW1] = cwe[:, 1] - 1.0
            vec[:, V_CW2] = cwe[:, 2]
            vec[:, V_LN0W] = np.asarray(inputs["ln0_w"], np.float32)
            vec[:, V_LN0B] = np.asarray(inputs["ln0_b"], np.float32)
            m[f"vecs{l}"] = vec
            # 0.5 fold: g is computed as x*(1+tanh(x/2)) = 2*silu(x)
            lnx = np.zeros((128, 1024), np.float32)
            lnx[:, 0:512] = 0.5 * W["lnx_w"][None, :]
            lnx[:, 512:1024] = 0.5 * W["lnx_b"][None, :]
            m[f"lnx{l}"] = lnx.astype(bf16)
            u = W["time_faaaa"].reshape(C)
            hmu = np.zeros((C, 8), np.float32)
            for h in range(H):
                hmu[h * HN:(h + 1) * HN, h] = u[h * HN:(h + 1) * HN]
            m[f"hmu{l}"] = hmu
        in_maps.append(m)
    return in_maps


def kernel(**inputs):
    if "nc" not in _CACHE:
        _CACHE["nc"] = _build(dbg=False)
    nc = _CACHE["nc"]
    in_maps = _host_inputs(inputs)
    res = run_bass_kernel_spmd(nc, in_maps, list(range(8)),
                               tmpdir=_CACHE.get("tmpdir"))
    _CACHE["last_results"] = res
    out = np.empty((B, T, C), np.float32)
    for b in range(B):
        out[b] = res.results[b]["xout"].T
    return out


if __name__ == "__main__":
    rng = np.random.default_rng(0)
    demo = None
